# revision 55
# baseline (speedup 1.0000x reference)
"""Trainium2 Bass kernel for nn_HGNNEncoder (DMPNN + global bond attention).

Sharding: data-parallel over bonds/atoms/molecules across 8 NeuronCores.
The global bond-bond attention stays exact: each core computes its 1024-row
slice of the 8192x8192 attention (bf16 QK^T, exp on ScalarE with the 1/16
scale folded in, bf16 P@[V|1] with fused row-sum column, normalized
post-hoc).  Between iterations each core computes its K^T / V column shard
locally and the shards are exchanged with half-sized bf16 AllGathers (so
attention on first-half columns starts while the second half is in flight);
the natural-layout message is AllGathered in bf16 for the DMPNN row-gathers,
which run on GpSimd fully overlapped with the attention.
"""

import numpy as np

import concourse.bass as bass
import concourse.bacc as bacc
import concourse.mybir as mybir
import concourse.tile as tile
from concourse.bass_utils import run_bass_kernel_spmd

NC = 8
B, NA, MNB = 8192, 4096, 6
H = 256
F = 133
FD = 147
BS = B // NC          # 1024 bonds per core
AS = NA // NC         # 512 atoms per core
APM = 64              # atoms per molecule
MPC = AS // APM       # 8 molecules per core
NITER = 3

f32 = mybir.dt.float32
f32r = mybir.dt.float32r
bf16 = mybir.dt.bfloat16
AF = mybir.ActivationFunctionType
ALU = mybir.AluOpType
AX = mybir.AxisListType

SCALE_BOND = float(1.0 / np.sqrt(np.float32(H)))
SCALE_ATOM = float(1.0 / np.sqrt(np.float32(F)))

# flash-loop column-block order
J_ORDER = list(range(B // 128))

_CACHE = {}


def _build():
    nc = bacc.Bacc("TRN2", target_bir_lowering=False, debug=False, num_devices=NC)

    def inp(name, shape, dt=f32):
        return nc.dram_tensor(name, list(shape), dt, kind="ExternalInput")

    fbT = inp("fbT", [H, BS])
    faT = inp("faT", [F, AS])
    fa = inp("fa", [AS, F])
    wq_a = inp("wq_a", [F, F]); wk_a = inp("wk_a", [F, F]); wv_a = inp("wv_a", [F, F])
    ln_g = inp("ln_g", [1, F]); ln_b = inp("ln_b", [1, F])
    wi = inp("wi", [H, H])
    wh = inp("wh", [H, H]); wq = inp("wq", [H, H]); wk = inp("wk", [H, H])
    wv = inp("wv", [H, H]); wa = inp("wa", [H, H])
    wal1 = inp("wal1", [H, 1]); wal2 = inp("wal2", [H, 1]); walb = inp("walb", [1, 1])
    wo_f = inp("wo_f", [F, H]); wo_m = inp("wo_m", [H, H]); wo_b = inp("wo_b", [1, H])
    wb_w = inp("wb_w", [H, H]); wb_b = inp("wb_b", [1, H])
    ident_in = inp("ident", [128, 128])
    onesr_in = inp("onesr", [1, 128])
    onesc_in = inp("onesc", [128, 1])
    nb_idx = inp("nb_idx", [128, BS // 128, 7], mybir.dt.int32)
    ro_idx = inp("ro_idx", [128, AS // 128, MNB], mybir.dt.int32)
    out_mol = nc.dram_tensor("mol_out", [MPC, H], f32, kind="ExternalOutput")

    with tile.TileContext(nc) as tc, \
         tc.tile_pool(name="persist", bufs=1) as per, \
         tc.tile_pool(name="dram", bufs=1, space="DRAM") as dram:

        def load_round(pool, src_ap, shape, name, dt=f32r, raw_pool=None):
            raw = (raw_pool or pool).tile(list(shape), f32, name=name + "_raw")
            nc.sync.dma_start(raw[:], src_ap)
            if dt == f32:
                return raw
            t = pool.tile(list(shape), dt, name=name)
            nc.vector.tensor_copy(t[:], raw[:])
            return t

        # ---------------- persistent weights ----------------
        with tc.tile_pool(name="raws", bufs=1) as raws:
            def loadw(src, name, cols=H):
                return load_round(per, src[:].rearrange("(t p) h -> p t h", p=128),
                                  [128, 2, cols], name, raw_pool=raws)
            Wi = loadw(wi, "Wi"); Wh = loadw(wh, "Wh"); Wq = loadw(wq, "Wq")
            Wk = loadw(wk, "Wk"); Wv = loadw(wv, "Wv"); Wa = loadw(wa, "Wa")
            Wom = loadw(wo_m, "Wom"); Wbw = loadw(wb_w, "Wbw")
            Wal1 = loadw(wal1, "Wal1", cols=1); Wal2 = loadw(wal2, "Wal2", cols=1)
            Wof_h = load_round(per, wo_f[0:128, :], [128, H], "Wof_h", raw_pool=raws)
            Wof_l = load_round(per, wo_f[128:F, :], [5, H], "Wof_l", raw_pool=raws)
            Wob = load_round(per, wo_b[:], [1, H], "Wob", raw_pool=raws)
            Wbb = load_round(per, wb_b[:], [1, H], "Wbb", raw_pool=raws)
            Ident = load_round(per, ident_in[:], [128, 128], "Ident", dt=f32)
            IdentB = load_round(per, ident_in[:], [128, 128], "IdentB", dt=bf16,
                                raw_pool=raws)
            Onesr = load_round(per, onesr_in[:], [1, 128], "Onesr", raw_pool=raws)
            OnesC = load_round(per, onesc_in[:], [128, 1], "OnesC", raw_pool=raws)
            Walb = load_round(per, walb[:], [1, 1], "Walb", dt=f32)
            OnesrF = load_round(per, onesr_in[:], [1, 128], "OnesrF", dt=f32)
            NbIdx = per.tile([128, BS // 128, 7], mybir.dt.int32, name="NbIdx")
            nc.sync.dma_start(NbIdx[:], nb_idx[:])
            RoIdx = per.tile([128, AS // 128, MNB], mybir.dt.int32, name="RoIdx")
            nc.sync.dma_start(RoIdx[:], ro_idx[:])


        # persistent activations
        InpT = per.tile([128, 2, BS], f32r, name="InpT")
        KT = per.tile([128, 2, B], bf16, name="KT")
        Vb = per.tile([128, B // 128, H + 1], bf16, name="Vb")
        QT = per.tile([128, 2, BS], bf16, name="QT")
        MsgTx = per.tile([128, 2, BS], f32r, name="MsgTx")   # local shard msgT
        KTl = per.tile([128, 2, BS], bf16, name="KTl")       # local K^T shard
        Vl = per.tile([128, BS // 128, H + 1], bf16, name="Vl")  # local V shard
        FeT_h = per.tile([128, AS], f32r, name="FeT_h")
        FeT_l = per.tile([5, AS], f32r, name="FeT_l")
        nc.vector.memset(Vl[:], 1.0)   # ones column persists across iters

        # packed per-core attention payload: [K^T 256x1024 | V 1024x257];
        # the natural-layout message travels in its own (later) AllGather
        KT_OFF = 0
        KT_SZ = H * BS                    # 262144
        V_OFF = KT_SZ                     # 262144
        V_SZ = BS * (H + 1)               # 263168
        PKT = V_OFF + V_SZ                # 525312
        pkt_in = [dram.tile([PKT], bf16, name=f"pkt_in{t}") for t in range(NITER)]
        pkt_out = [dram.tile([NC * PKT], bf16, name=f"pkt_out{t}",
                             addr_space="Shared") for t in range(NITER)]
        agn_ins = [dram.tile([BS, H], bf16, name=f"agn_in{t}")
                   for t in range(NITER + 1)]
        agn_outs = [dram.tile([B, H], bf16, name=f"agn_out{t}",
                              addr_space="Shared") for t in range(NITER + 1)]
        RG = [list(range(NC))]

        def emit_exchange(t, psum_pool, tag, natc_pool):
            """K^T/V shards + natural msg from MsgTx, packed, one AllGather."""
            for half in range(2):
                for ht in range(2):
                    ps = psum_pool.tile([128, BS // 2], f32, name=tag)
                    for kt in range(2):
                        nc.tensor.matmul(ps[:], Wk[:, kt, bass.ts(ht, 128)],
                                         MsgTx[:, kt, bass.ts(half, BS // 2)],
                                         start=(kt == 0), stop=(kt == 1))
                    nc.vector.tensor_copy(KTl[:, ht, bass.ts(half, BS // 2)],
                                          ps[:])
            nc.sync.dma_start(
                pkt_in[t][KT_OFF:KT_OFF + KT_SZ]
                .rearrange("(p t i) -> p t i", p=128, t=2),
                KTl[:])
            for c in range(BS // 128):
                ps = psum_pool.tile([128, H], f32, name=tag)
                for kt in range(2):
                    nc.tensor.matmul(ps[:], MsgTx[:, kt, bass.ts(c, 128)],
                                     Wv[:, kt, :],
                                     start=(kt == 0), stop=(kt == 1))
                nc.vector.tensor_copy(Vl[:, c, 0:H], ps[:])
            nc.sync.dma_start(
                pkt_in[t][V_OFF:V_OFF + V_SZ]
                .rearrange("(c p h) -> p c h", p=128, c=BS // 128),
                Vl[:])
            nc.gpsimd.collective_compute(
                "AllGather", ALU.bypass, RG,
                ins=[pkt_in[t].opt()], outs=[pkt_out[t].opt()])
            # natural layout via PE transposes, DMA per 128-bond block;
            # its AllGather (feeding the dmpnn gathers) goes out second
            for c in range(BS // 128):
                psn = psum_pool.tile([128, H], f32, name=tag)
                for ht in range(2):
                    nc.tensor.transpose(
                        psn[:, bass.ts(ht, 128)],
                        MsgTx[:, ht, bass.ts(c, 128)].bitcast(f32), Ident[:])
                natc = natc_pool.tile([128, H], bf16, name="natc")
                nc.vector.tensor_copy(natc[:], psn[:])
                nc.sync.dma_start(agn_ins[t][bass.ts(c, 128), :], natc[:])
            nc.gpsimd.collective_compute(
                "AllGather", ALU.bypass, RG,
                ins=[agn_ins[t].opt()], outs=[agn_outs[t].opt()])

        # ============ Phase B: message_0 = relu(f_bonds @ Wi) ==============
        with tc.tile_pool(name="pb", bufs=1) as pb, \
             tc.tile_pool(name="pb2", bufs=2) as pb2, \
             tc.tile_pool(name="pb_ps", bufs=4, space="PSUM") as pbps:
            FbT = load_round(pb, fbT[:].rearrange("(t p) i -> p t i", p=128),
                             [128, 2, BS], "FbT")
            for ht in range(2):
                for ch in range(BS // 512):
                    ps = pbps.tile([128, 512], f32, name="ps")
                    for kt in range(2):
                        nc.tensor.matmul(ps[:], Wi[:, kt, bass.ts(ht, 128)],
                                         FbT[:, kt, bass.ts(ch, 512)],
                                         start=(kt == 0), stop=(kt == 1))
                    nc.vector.tensor_copy(InpT[:, ht, bass.ts(ch, 512)], ps[:])
                    nc.scalar.activation(MsgTx[:, ht, bass.ts(ch, 512)], ps[:],
                                         AF.Relu)
            emit_exchange(0, pbps, "ps", pb2)

        # ============ Phase A: per-molecule atom self-attention ============
        with tc.tile_pool(name="pa", bufs=1) as pa, \
             tc.tile_pool(name="pa2", bufs=2) as pa2, \
             tc.tile_pool(name="pa_ps", bufs=4, space="PSUM") as paps:
            WqA_h = load_round(pa, wq_a[0:128, :], [128, F], "WqA_h", dt=f32)
            WqA_l = load_round(pa, wq_a[128:F, :], [5, F], "WqA_l", dt=f32)
            WkA_h = load_round(pa, wk_a[0:128, :], [128, F], "WkA_h", dt=f32)
            WkA_l = load_round(pa, wk_a[128:F, :], [5, F], "WkA_l", dt=f32)
            WvA_h = load_round(pa, wv_a[0:128, :], [128, F], "WvA_h", dt=f32)
            WvA_l = load_round(pa, wv_a[128:F, :], [5, F], "WvA_l", dt=f32)
            LnG = load_round(pa, ln_g[:], [1, F], "LnG", dt=f32)
            LnB = load_round(pa, ln_b[:], [1, F], "LnB", dt=f32)
            FaT_h = load_round(pa, faT[0:128, :], [128, AS], "FaT_h", dt=f32)
            FaT_l = load_round(pa, faT[128:F, :], [5, AS], "FaT_l", dt=f32)
            XN = pa.tile([64, MPC, F], f32, name="XN")
            nc.sync.dma_start(XN[:], fa[:].rearrange("(m a) f -> a m f", a=64))
            GB = pa.tile([64, F], f32, name="GB")
            BB = pa.tile([64, F], f32, name="BB")
            for bc_src, bc_dst in ((LnG, GB), (LnB, BB)):
                ps = paps.tile([64, F], f32, name="ps")
                nc.tensor.matmul(ps[:], OnesrF[:, 0:64], bc_src[:],
                                 start=True, stop=True)
                nc.vector.tensor_copy(bc_dst[:], ps[:])

            def mm133(dst, lhs_pair, rhs_pair, n):
                ps = paps.tile([dst.shape[0], n], f32, name="ps")
                nc.tensor.matmul(ps[:], lhs_pair[0], rhs_pair[0],
                                 start=True, stop=False)
                nc.tensor.matmul(ps[:], lhs_pair[1], rhs_pair[1],
                                 start=False, stop=True)
                nc.vector.tensor_copy(dst, ps[:])

            QTa_h = pa.tile([128, AS], f32, name="QTa_h")
            QTa_l = pa.tile([5, AS], f32, name="QTa_l")
            KTa_h = pa.tile([128, AS], f32, name="KTa_h")
            KTa_l = pa.tile([5, AS], f32, name="KTa_l")
            mm133(QTa_h[:], (WqA_h[:, 0:128], WqA_l[:, 0:128]),
                  (FaT_h[:], FaT_l[:]), AS)
            mm133(QTa_l[:], (WqA_h[:, 128:F], WqA_l[:, 128:F]),
                  (FaT_h[:], FaT_l[:]), AS)
            mm133(KTa_h[:], (WkA_h[:, 0:128], WkA_l[:, 0:128]),
                  (FaT_h[:], FaT_l[:]), AS)
            mm133(KTa_l[:], (WkA_h[:, 128:F], WkA_l[:, 128:F]),
                  (FaT_h[:], FaT_l[:]), AS)
            VN = pa.tile([64, MPC, F], f32, name="VN")
            for m in range(MPC):
                mm133(VN[:, m, :],
                      (FaT_h[:, bass.ts(m, 64)], FaT_l[:, bass.ts(m, 64)]),
                      (WvA_h[:], WvA_l[:]), F)
            E = pa.tile([64, MPC, 64], f32, name="E")
            for m in range(MPC):
                ps = paps.tile([64, 64], f32, name="ps")
                nc.tensor.matmul(ps[:], QTa_h[:, bass.ts(m, 64)],
                                 KTa_h[:, bass.ts(m, 64)], start=True, stop=False)
                nc.tensor.matmul(ps[:], QTa_l[:, bass.ts(m, 64)],
                                 KTa_l[:, bass.ts(m, 64)], start=False, stop=True)
                nc.scalar.activation(E[:, m, :], ps[:], AF.Exp, scale=SCALE_ATOM)
            SumsA = pa.tile([64, MPC, 1], f32, name="SumsA")
            RS = pa.tile([64, MPC, 1], f32, name="RS")
            nc.vector.tensor_reduce(SumsA[:], E[:], axis=AX.X, op=ALU.add)
            nc.vector.reciprocal(RS[:], SumsA[:])
            AttnN = pa.tile([64, MPC, F], f32, name="AttnN")
            for m in range(MPC):
                pst = paps.tile([64, 64], f32, name="ps")
                nc.tensor.transpose(pst[:], E[:, m, :], Ident[0:64, 0:64])
                ET = pa2.tile([64, 64], f32, name="ET")
                nc.vector.tensor_copy(ET[:], pst[:])
                ps = paps.tile([64, F], f32, name="ps")
                nc.tensor.matmul(ps[:], ET[:], VN[:, m, :], start=True, stop=True)
                nc.vector.tensor_scalar_mul(AttnN[:, m, :], ps[:], RS[:, m, :])
            SumX = pa.tile([64, MPC, F], f32, name="SumX")
            nc.vector.tensor_tensor(SumX[:], XN[:], AttnN[:], op=ALU.add)
            Mu = pa.tile([64, MPC, 1], f32, name="Mu")
            nc.vector.tensor_reduce(Mu[:], SumX[:], axis=AX.X, op=ALU.add)
            nc.vector.tensor_scalar_mul(Mu[:], Mu[:], 1.0 / F)
            XC = pa.tile([64, MPC, F], f32, name="XC")
            for m in range(MPC):
                nc.vector.tensor_scalar_sub(XC[:, m, :], SumX[:, m, :], Mu[:, m, :])
            SQ = pa.tile([64, MPC, F], f32, name="SQ")
            nc.vector.tensor_tensor(SQ[:], XC[:], XC[:], op=ALU.mult)
            Var = pa.tile([64, MPC, 1], f32, name="Var")
            nc.vector.tensor_reduce(Var[:], SQ[:], axis=AX.X, op=ALU.add)
            Std = pa.tile([64, MPC, 1], f32, name="Std")
            EpsT = pa.tile([64, 1], f32, name="EpsT")
            nc.vector.memset(EpsT[:], 1e-5)
            nc.scalar.activation(Std[:], Var[:], AF.Sqrt, scale=1.0 / F,
                                 bias=EpsT[:])
            RStd = pa.tile([64, MPC, 1], f32, name="RStd")
            nc.vector.reciprocal(RStd[:], Std[:])
            FeN = pa.tile([64, MPC, F], f32, name="FeN")
            for m in range(MPC):
                nc.vector.tensor_scalar_mul(XC[:, m, :], XC[:, m, :], RStd[:, m, :])
                nc.vector.tensor_tensor(XC[:, m, :], XC[:, m, :], GB[:], op=ALU.mult)
                nc.vector.tensor_tensor(FeN[:, m, :], XC[:, m, :], BB[:], op=ALU.add)
            for m in range(MPC):
                ps1 = paps.tile([128, 64], f32, name="ps")
                nc.tensor.transpose(ps1[:], FeN[:, m, 0:128], Ident[0:64, 0:64])
                nc.vector.tensor_copy(FeT_h[:, bass.ts(m, 64)], ps1[:])
                ps2 = paps.tile([5, 64], f32, name="ps")
                nc.tensor.transpose(ps2[:], FeN[:, m, 128:F], Ident[0:64, 0:64])
                nc.vector.tensor_copy(FeT_l[:, bass.ts(m, 64)], ps2[:])

        # ============ Phase C: 3 message-passing iterations ================
        for it in range(NITER):
            mnall = agn_outs[it][:]
            with tc.tile_pool(name=f"pc{it}", bufs=1) as pc, \
                 tc.tile_pool(name=f"pm{it}", bufs=2) as pmt, \
                 tc.tile_pool(name=f"pcA{it}", bufs=2, space="PSUM") as ppsA, \
                 tc.tile_pool(name=f"pcS{it}", bufs=2, space="PSUM") as ppsS, \
                 tc.tile_pool(name=f"pcP{it}", bufs=1, space="PSUM") as ppsP:

                # ---- QT from the local shard (no collective dependency)
                for ch in range(2):
                    for ht in range(2):
                        ps = ppsA.tile([128, 512], f32, name="psA")
                        for kt in range(2):
                            nc.tensor.matmul(ps[:], Wq[:, kt, bass.ts(ht, 128)],
                                             MsgTx[:, kt, bass.ts(ch, 512)],
                                             start=(kt == 0), stop=(kt == 1))
                        nc.vector.tensor_copy(QT[:, ht, bass.ts(ch, 512)], ps[:])

                # ---- load gathered K^T / V shards from the packed payload
                for s in range(NC):
                    nc.sync.dma_start(
                        KT[:, :, s * BS:(s + 1) * BS],
                        pkt_out[it][s * PKT + KT_OFF:s * PKT + KT_OFF + KT_SZ]
                        .rearrange("(p t i) -> p t i", p=128, t=2))
                    nc.sync.dma_start(
                        Vb[:, s * 8:(s + 1) * 8, :],
                        pkt_out[it][s * PKT + V_OFF:s * PKT + V_OFF + V_SZ]
                        .rearrange("(c p h) -> p c h", p=128, c=BS // 128))

                # ---- issue dmpnn gathers early (gpsimd) so they overlap the
                # ---- attention below (tensor engine)
                Tn = pc.tile([128, BS // 128, H], bf16, name="Tn")
                for c in range(BS // 128):
                    nc.gpsimd.indirect_dma_start(
                        out=Tn[:, c, :], out_offset=None, in_=mnall,
                        in_offset=bass.IndirectOffsetOnAxis(
                            ap=NbIdx[:, c, 0:1], axis=0))
                nc.vector.tensor_scalar_mul(Tn[:], Tn[:], -1.0)
                for j in range(1, 7):
                    for c in range(BS // 128):
                        nc.gpsimd.indirect_dma_start(
                            out=Tn[:, c, :], out_offset=None, in_=mnall,
                            in_offset=bass.IndirectOffsetOnAxis(
                                ap=NbIdx[:, c, j:j + 1], axis=0),
                            compute_op=ALU.add)

                # ---- attention (rows = shard), flash-style over j blocks
                patt = tc.alloc_tile_pool(name=f"patt{it}", bufs=1)
                AttA = patt.tile([128, BS // 128, H], f32r, name="AttA")
                for ic in range(BS // 512):
                    pvp = ppsP.tile([128, 4, 512], f32, name="psP")
                    prev = None
                    for jx in range(B // 128 + 1):
                        if jx < B // 128:
                            j = J_ORDER[jx]
                            sp = ppsS.tile([128, 512], f32, name="psS")
                            for kt in range(2):
                                nc.tensor.matmul(sp[:], KT[:, kt, bass.ts(j, 128)],
                                                 QT[:, kt, bass.ts(ic, 512)],
                                                 start=(kt == 0), stop=(kt == 1))
                            pt = pmt.tile([128, 512], bf16, name="PT")
                            nc.scalar.activation(pt[:], sp[:], AF.Exp,
                                                 scale=SCALE_BOND)
                        if prev is not None:
                            pj, ppt = prev
                            for isub in range(4):
                                nc.tensor.matmul(
                                    pvp[:, isub, 0:H + 1],
                                    ppt[:, bass.ts(isub, 128)],
                                    Vb[:, pj, 0:H + 1],
                                    start=(pj == J_ORDER[0]),
                                    stop=(pj == J_ORDER[-1]))
                        if jx < B // 128:
                            prev = (j, pt)
                    for isub in range(4):
                        rec = pmt.tile([128, 1], f32, name="rec")
                        nc.vector.reciprocal(rec[:], pvp[:, isub, H:H + 1])
                        nc.vector.tensor_scalar_mul(
                            AttA[:, ic * 4 + isub, :], pvp[:, isub, 0:H], rec[:])

                # ---- dmpnn path: transpose + Wh.  The transposes read IdentF,
                # a copy of the identity carrying an artificial zero-valued
                # dependency on the LAST attention output, so the scheduler
                # cannot hoist this gather-dependent block into the attention
                # stream (that stalls the tensor engine on the slow gathers).
                Zf = pmt.tile([128, 1], f32, name="Zf")
                nc.vector.tensor_scalar_mul(
                    Zf[:], AttA[:, BS // 128 - 1, 0:1].bitcast(f32), 0.0)
                IdentF = pc.tile([128, 128], bf16, name="IdentF")
                nc.vector.tensor_scalar_add(IdentF[:], IdentB[:], Zf[:])
                TT = pc.tile([128, 2, BS], f32r, name="TT")
                DmT = pc.tile([128, 2, BS], f32r, name="DmT")
                for ht in range(2):
                    for half in range(2):
                        pst = ppsA.tile([128, 512], bf16, name="psA")
                        for cc in range(4):
                            c = half * 4 + cc
                            nc.tensor.transpose(
                                pst[:, bass.ts(cc, 128)],
                                Tn[:, c, bass.ts(ht, 128)],
                                IdentF[:])
                        nc.vector.tensor_copy(
                            TT[:, ht, bass.ts(half, 512)], pst[:])
                for ht in range(2):
                    for ch in range(2):
                        ps = ppsA.tile([128, 512], f32, name="psA")
                        for kt in range(2):
                            nc.tensor.matmul(
                                ps[:], Wh[:, kt, bass.ts(ht, 128)],
                                TT[:, kt, bass.ts(ch, 512)],
                                start=(kt == 0), stop=(kt == 1))
                        nc.vector.tensor_copy(DmT[:, ht, bass.ts(ch, 512)],
                                              ps[:])

                # ---- att_msgT, alpha, combine
                if True:
                    pt2 = patt
                    AtT = pt2.tile([128, 2, BS], f32r, name="AtT")
                    for ht in range(2):
                        for half in range(2):
                            pst = ppsA.tile([128, 512], f32, name="psA")
                            for cc in range(4):
                                c = half * 4 + cc
                                nc.tensor.transpose(
                                    pst[:, bass.ts(cc, 128)],
                                    AttA[:, c, bass.ts(ht, 128)].bitcast(f32),
                                    Ident[:])
                            nc.vector.tensor_copy(
                                AtT[:, ht, bass.ts(half, 512)], pst[:])
                    AmT = pt2.tile([128, 2, BS], f32r, name="AmT")
                    for ht in range(2):
                        for ch in range(2):
                            ps = ppsA.tile([128, 512], f32, name="psA")
                            for kt in range(2):
                                nc.tensor.matmul(ps[:], Wa[:, kt, bass.ts(ht, 128)],
                                                 AtT[:, kt, bass.ts(ch, 512)],
                                                 start=(kt == 0), stop=(kt == 1))
                            nc.vector.tensor_copy(AmT[:, ht, bass.ts(ch, 512)],
                                                  ps[:])

                Alp = pc.tile([1, BS], f32r, name="Alp")
                for ch in range(2):
                    ps = ppsA.tile([1, 512], f32, name="psA")
                    nc.tensor.matmul(ps[:], Wal1[:, 0, :],
                                     DmT[:, 0, bass.ts(ch, 512)],
                                     start=True, stop=False)
                    nc.tensor.matmul(ps[:], Wal1[:, 1, :],
                                     DmT[:, 1, bass.ts(ch, 512)],
                                     start=False, stop=False)
                    nc.tensor.matmul(ps[:], Wal2[:, 0, :],
                                     AmT[:, 0, bass.ts(ch, 512)],
                                     start=False, stop=False)
                    nc.tensor.matmul(ps[:], Wal2[:, 1, :],
                                     AmT[:, 1, bass.ts(ch, 512)],
                                     start=False, stop=True)
                    nc.scalar.activation(Alp[:, bass.ts(ch, 512)], ps[:],
                                         AF.Sigmoid, bias=Walb[:])
                AlB = pc.tile([128, BS], f32, name="AlB")
                for ch in range(2):
                    ps = ppsA.tile([128, 512], f32, name="psA")
                    nc.tensor.matmul(ps[:], Onesr[:], Alp[:, bass.ts(ch, 512)],
                                     start=True, stop=True)
                    nc.vector.tensor_copy(AlB[:, bass.ts(ch, 512)], ps[:])

                D1 = pc.tile([128, BS], f32, name="D1")
                for ht in range(2):
                    nc.vector.tensor_tensor(D1[:], DmT[:, ht, :], AmT[:, ht, :],
                                            op=ALU.subtract)
                    nc.vector.tensor_tensor(D1[:], D1[:], AlB[:], op=ALU.mult)
                    nc.vector.tensor_tensor(D1[:], D1[:], AmT[:, ht, :], op=ALU.add)
                    nc.vector.tensor_tensor(D1[:], D1[:], InpT[:, ht, :], op=ALU.add)
                    nc.scalar.activation(MsgTx[:, ht, :], D1[:], AF.Relu)
                patt.release()
                if it + 1 < NITER:
                    emit_exchange(it + 1, ppsA, "psA", pmt)
                else:
                    # final boundary: only the natural msg for the readout
                    for c in range(BS // 128):
                        psn = ppsA.tile([128, H], f32, name="psA")
                        for ht in range(2):
                            nc.tensor.transpose(
                                psn[:, bass.ts(ht, 128)],
                                MsgTx[:, ht, bass.ts(c, 128)].bitcast(f32),
                                Ident[:])
                        natc = pmt.tile([128, H], bf16, name="natc")
                        nc.vector.tensor_copy(natc[:], psn[:])
                        nc.sync.dma_start(agn_ins[NITER][bass.ts(c, 128), :],
                                          natc[:])
                    nc.gpsimd.collective_compute(
                        "AllGather", ALU.bypass, RG,
                        ins=[agn_ins[NITER].opt()], outs=[agn_outs[NITER].opt()])

        # ============ Readout + per-molecule pooling =======================
        with tc.tile_pool(name="pd", bufs=1) as pd, \
             tc.tile_pool(name="pd2", bufs=2) as pd2, \
             tc.tile_pool(name="pd_ps", bufs=4, space="PSUM") as pdps:
            AmN = pd.tile([128, AS // 128, H], bf16, name="AmN")
            for c in range(AS // 128):
                nc.gpsimd.indirect_dma_start(
                    out=AmN[:, c, :], out_offset=None, in_=agn_outs[NITER][:],
                    in_offset=bass.IndirectOffsetOnAxis(
                        ap=RoIdx[:, c, 0:1], axis=0))
            for j in range(1, MNB):
                for c in range(AS // 128):
                    nc.gpsimd.indirect_dma_start(
                        out=AmN[:, c, :], out_offset=None, in_=agn_outs[NITER][:],
                        in_offset=bass.IndirectOffsetOnAxis(
                            ap=RoIdx[:, c, j:j + 1], axis=0),
                        compute_op=ALU.add)
            AmT2 = pd.tile([128, 2, AS], f32r, name="AmT2")
            for ht in range(2):
                pst = pdps.tile([128, AS], bf16, name="ps")
                for c in range(AS // 128):
                    nc.tensor.transpose(pst[:, bass.ts(c, 128)],
                                        AmN[:, c, bass.ts(ht, 128)],
                                        IdentB[:])
                nc.vector.tensor_copy(AmT2[:, ht, :], pst[:])
            Hm = pd.tile([128, AS // 128, H], f32r, name="Hm")
            for c in range(AS // 128):
                ps = pdps.tile([128, H], f32, name="ps")
                nc.tensor.matmul(ps[:], FeT_h[:, bass.ts(c, 128)], Wof_h[:],
                                 start=True, stop=False)
                nc.tensor.matmul(ps[:], FeT_l[:, bass.ts(c, 128)], Wof_l[:],
                                 start=False, stop=False)
                for kt in range(2):
                    nc.tensor.matmul(ps[:], AmT2[:, kt, bass.ts(c, 128)],
                                     Wom[:, kt, :], start=False, stop=False)
                nc.tensor.matmul(ps[:], Onesr[:, 0:128], Wob[:],
                                 start=False, stop=True)
                nc.scalar.activation(Hm[:, c, :], ps[:], AF.Relu)
            HmT = pd.tile([128, 2, AS], f32r, name="HmT")
            for ht in range(2):
                pst = pdps.tile([128, AS], f32, name="ps")
                for c in range(AS // 128):
                    nc.tensor.transpose(pst[:, bass.ts(c, 128)],
                                        Hm[:, c, bass.ts(ht, 128)].bitcast(f32),
                                        Ident[:])
                nc.vector.tensor_copy(HmT[:, ht, :], pst[:])
            # hm in per-molecule base-0 layout via SBUF->SBUF DMA
            HmM = pd.tile([64, MPC, H], f32r, name="HmM")
            nc.sync.dma_start(HmM[:, 0:MPC:2, :], Hm[0:64, :, :])
            nc.sync.dma_start(HmM[:, 1:MPC:2, :], Hm[64:128, :, :])
            T2T = pd.tile([128, 2, AS], f32r, name="T2T")
            for ht in range(2):
                ps = pdps.tile([128, AS], f32, name="ps")
                for kt in range(2):
                    nc.tensor.matmul(ps[:], Wa[:, kt, bass.ts(ht, 128)],
                                     HmT[:, kt, :], start=(kt == 0), stop=(kt == 1))
                nc.vector.tensor_copy(T2T[:, ht, :], ps[:])
            SC2 = pd.tile([64, MPC, 64], f32, name="SC2")
            for m in range(MPC):
                ps = pdps.tile([64, 64], f32, name="ps")
                for kt in range(2):
                    nc.tensor.matmul(ps[:], T2T[:, kt, bass.ts(m, 64)],
                                     HmT[:, kt, bass.ts(m, 64)],
                                     start=(kt == 0), stop=(kt == 1))
                nc.vector.tensor_copy(SC2[:, m, :], ps[:])
            Mx2 = pd.tile([64, MPC, 1], f32, name="Mx2")
            nc.vector.tensor_reduce(Mx2[:], SC2[:], axis=AX.X, op=ALU.max)
            NMx2 = pd.tile([64, MPC, 1], f32, name="NMx2")
            nc.vector.tensor_scalar_mul(NMx2[:], Mx2[:], -1.0)
            E2 = pd.tile([64, MPC, 64], f32, name="E2")
            for m in range(MPC):
                nc.scalar.activation(E2[:, m, :], SC2[:, m, :], AF.Exp,
                                     bias=NMx2[:, m, :])
            Sum2 = pd.tile([64, MPC, 1], f32, name="Sum2")
            RS2 = pd.tile([64, MPC, 1], f32, name="RS2")
            nc.vector.tensor_reduce(Sum2[:], E2[:], axis=AX.X, op=ALU.add)
            nc.vector.reciprocal(RS2[:], Sum2[:])
            BB2 = pd.tile([64, H], f32, name="BB2")
            psbb = pdps.tile([64, H], f32, name="ps")
            nc.tensor.matmul(psbb[:], Onesr[:, 0:64], Wbb[:], start=True, stop=True)
            nc.vector.tensor_copy(BB2[:], psbb[:])
            OutS = pd.tile([1, MPC, H], f32, name="OutS")
            for m in range(MPC):
                pst = pdps.tile([64, 64], f32, name="ps")
                nc.tensor.transpose(pst[:], E2[:, m, :], Ident[0:64, 0:64])
                E2T = pd2.tile([64, 64], f32r, name="E2T")
                nc.vector.tensor_copy(E2T[:], pst[:])
                UT = pd2.tile([128, 2, 64], f32r, name="UT")
                for hs in range(2):
                    psu = pdps.tile([128, 64], f32, name="ps")
                    nc.tensor.matmul(psu[:], HmM[:, m, bass.ts(hs, 128)], E2T[:],
                                     start=True, stop=True)
                    nc.vector.tensor_copy(UT[:, hs, :], psu[:])
                psb = pdps.tile([64, H], f32, name="ps")
                for kt in range(2):
                    nc.tensor.matmul(psb[:], UT[:, kt, :], Wbw[:, kt, :],
                                     start=(kt == 0), stop=(kt == 1))
                AH = pd2.tile([64, H], f32, name="AH")
                nc.vector.tensor_scalar_mul(AH[:], psb[:], RS2[:, m, :])
                nc.vector.tensor_tensor(AH[:], AH[:], BB2[:], op=ALU.add)
                nc.vector.tensor_scalar_max(AH[:], AH[:], 0.0)
                XS = pd2.tile([64, H], f32r, name="XS")
                nc.vector.tensor_tensor(XS[:], AH[:], HmM[:, m, :], op=ALU.add)
                psm = pdps.tile([1, H], f32, name="ps")
                nc.tensor.matmul(psm[:], OnesC[0:64, :], XS[:],
                                 start=True, stop=True)
                nc.vector.tensor_scalar_mul(OutS[:, m, :], psm[:], 1.0 / APM)
            nc.sync.dma_start(out_mol[:].rearrange("(o m) h -> o m h", o=1), OutS[:])

    nc.compile()
    return nc


def _host_prepare(inputs):
    f_atoms = np.asarray(inputs["f_atoms"], np.float32)
    f_bonds = np.asarray(inputs["f_bonds"], np.float32)
    a2b = np.asarray(inputs["a2b"]).astype(np.int64)
    b2a = np.asarray(inputs["b2a"]).astype(np.int64)
    b2revb = np.asarray(inputs["b2revb"]).astype(np.int64)

    fbp = np.zeros((B, H), np.float32)
    fbp[:, :FD] = f_bonds
    fbT_full = np.ascontiguousarray(fbp.T)
    faT_full = np.ascontiguousarray(f_atoms.T)

    W = {k: np.asarray(inputs[k], np.float32) for k in
         ("Wq_atom", "Wk_atom", "Wv_atom", "Wi", "Wh", "Wq", "Wk", "Wv", "Wa",
          "Walpha_w", "Wo_w", "Wb_w")}
    wi_p = np.zeros((H, H), np.float32)
    wi_p[:FD, :] = W["Wi"]

    base = dict(
        wq_a=W["Wq_atom"], wk_a=W["Wk_atom"], wv_a=W["Wv_atom"],
        ln_g=np.asarray(inputs["ln_g"], np.float32).reshape(1, F),
        ln_b=np.asarray(inputs["ln_b"], np.float32).reshape(1, F),
        wi=wi_p, wh=W["Wh"], wq=W["Wq"], wk=W["Wk"], wv=W["Wv"], wa=W["Wa"],
        wal1=np.ascontiguousarray(W["Walpha_w"][:H]),
        wal2=np.ascontiguousarray(W["Walpha_w"][H:]),
        walb=np.asarray(inputs["Walpha_b"], np.float32).reshape(1, 1),
        wo_f=np.ascontiguousarray(W["Wo_w"][:F]),
        wo_m=np.ascontiguousarray(W["Wo_w"][F:]),
        wo_b=np.asarray(inputs["Wo_b"], np.float32).reshape(1, H),
        wb_w=W["Wb_w"],
        wb_b=np.asarray(inputs["Wb_b"], np.float32).reshape(1, H),
        ident=np.eye(128, dtype=np.float32),
        onesr=np.ones((1, 128), np.float32),
        onesc=np.ones((128, 1), np.float32),
    )

    in_maps = []
    for c in range(NC):
        bonds = np.arange(c * BS, (c + 1) * BS)
        # [7, BS]: row 0 = b2revb (subtracted), rows 1..6 = a2b[b2a]
        terms = np.stack([b2revb[bonds]]
                         + [a2b[b2a[bonds], j] for j in range(MNB)])
        nb2 = np.ascontiguousarray(
            terms.reshape(7, BS // 128, 128).transpose(2, 1, 0)).astype(np.int32)
        atoms = np.arange(c * AS, (c + 1) * AS)
        ro = np.stack([a2b[atoms, j] for j in range(MNB)])  # [6, AS]
        ro2 = np.ascontiguousarray(
            ro.reshape(MNB, AS // 128, 128).transpose(2, 1, 0)).astype(np.int32)
        m = dict(base)
        m["fbT"] = np.ascontiguousarray(fbT_full[:, bonds])
        m["faT"] = np.ascontiguousarray(faT_full[:, atoms])
        m["fa"] = np.ascontiguousarray(f_atoms[atoms])
        m["nb_idx"] = nb2
        m["ro_idx"] = ro2
        in_maps.append(m)
    return in_maps


def kernel(**inputs):
    if "nc" not in _CACHE:
        _CACHE["nc"] = _build()
    nc = _CACHE["nc"]
    in_maps = _host_prepare(inputs)
    res = run_bass_kernel_spmd(nc, in_maps, core_ids=list(range(NC)))
    out = np.concatenate([res.results[c]["mol_out"] for c in range(NC)], 0)
    return np.ascontiguousarray(out.astype(np.float32))


# revision 59
# speedup vs baseline: 1.0194x; 1.0194x over previous
"""Trainium2 Bass kernel for nn_HGNNEncoder (DMPNN + global bond attention).

Sharding: data-parallel over bonds/atoms/molecules across 8 NeuronCores.
The global bond-bond attention stays exact: each core computes its 1024-row
slice of the 8192x8192 attention (bf16 QK^T, exp on ScalarE with the 1/16
scale folded in, bf16 P@[V|1] with fused row-sum column, normalized
post-hoc).  Between iterations each core computes its K^T / V column shard
locally and the shards are exchanged with half-sized bf16 AllGathers (so
attention on first-half columns starts while the second half is in flight);
the natural-layout message is AllGathered in bf16 for the DMPNN row-gathers,
which run on GpSimd fully overlapped with the attention.
"""

import numpy as np

import concourse.bass as bass
import concourse.bacc as bacc
import concourse.mybir as mybir
import concourse.tile as tile
from concourse.bass_utils import run_bass_kernel_spmd

NC = 8
B, NA, MNB = 8192, 4096, 6
H = 256
F = 133
FD = 147
BS = B // NC          # 1024 bonds per core
AS = NA // NC         # 512 atoms per core
APM = 64              # atoms per molecule
MPC = AS // APM       # 8 molecules per core
NITER = 3

f32 = mybir.dt.float32
f32r = mybir.dt.float32r
bf16 = mybir.dt.bfloat16
AF = mybir.ActivationFunctionType
ALU = mybir.AluOpType
AX = mybir.AxisListType

SCALE_BOND = float(1.0 / np.sqrt(np.float32(H)))
SCALE_ATOM = float(1.0 / np.sqrt(np.float32(F)))

# flash-loop column-block order: every core exchanges the first half of its
# bond shard first, so all first-half column blocks (j%8 < 4) arrive first
J_ORDER = [j for j in range(B // 128) if j % 8 < 4] + \
          [j for j in range(B // 128) if j % 8 >= 4]

_CACHE = {}


def _build():
    nc = bacc.Bacc("TRN2", target_bir_lowering=False, debug=False, num_devices=NC)

    def inp(name, shape, dt=f32):
        return nc.dram_tensor(name, list(shape), dt, kind="ExternalInput")

    fbT = inp("fbT", [H, BS])
    faT = inp("faT", [F, AS])
    fa = inp("fa", [AS, F])
    wq_a = inp("wq_a", [F, F]); wk_a = inp("wk_a", [F, F]); wv_a = inp("wv_a", [F, F])
    ln_g = inp("ln_g", [1, F]); ln_b = inp("ln_b", [1, F])
    wi = inp("wi", [H, H])
    wh = inp("wh", [H, H]); wq = inp("wq", [H, H]); wk = inp("wk", [H, H])
    wv = inp("wv", [H, H]); wa = inp("wa", [H, H])
    wal1 = inp("wal1", [H, 1]); wal2 = inp("wal2", [H, 1]); walb = inp("walb", [1, 1])
    wo_f = inp("wo_f", [F, H]); wo_m = inp("wo_m", [H, H]); wo_b = inp("wo_b", [1, H])
    wb_w = inp("wb_w", [H, H]); wb_b = inp("wb_b", [1, H])
    ident_in = inp("ident", [128, 128])
    onesr_in = inp("onesr", [1, 128])
    onesc_in = inp("onesc", [128, 1])
    nb_idx = inp("nb_idx", [128, BS // 128, 7], mybir.dt.int32)
    ro_idx = inp("ro_idx", [128, AS // 128, MNB], mybir.dt.int32)
    out_mol = nc.dram_tensor("mol_out", [MPC, H], f32, kind="ExternalOutput")

    with tile.TileContext(nc) as tc, \
         tc.tile_pool(name="persist", bufs=1) as per, \
         tc.tile_pool(name="dram", bufs=1, space="DRAM") as dram:

        def load_round(pool, src_ap, shape, name, dt=f32r, raw_pool=None):
            raw = (raw_pool or pool).tile(list(shape), f32, name=name + "_raw")
            nc.sync.dma_start(raw[:], src_ap)
            if dt == f32:
                return raw
            t = pool.tile(list(shape), dt, name=name)
            nc.vector.tensor_copy(t[:], raw[:])
            return t

        # ---------------- persistent weights ----------------
        with tc.tile_pool(name="raws", bufs=1) as raws:
            def loadw(src, name, cols=H):
                return load_round(per, src[:].rearrange("(t p) h -> p t h", p=128),
                                  [128, 2, cols], name, raw_pool=raws)
            Wi = loadw(wi, "Wi"); Wh = loadw(wh, "Wh"); Wq = loadw(wq, "Wq")
            Wk = loadw(wk, "Wk"); Wv = loadw(wv, "Wv"); Wa = loadw(wa, "Wa")
            Wom = loadw(wo_m, "Wom"); Wbw = loadw(wb_w, "Wbw")
            Wal1 = loadw(wal1, "Wal1", cols=1); Wal2 = loadw(wal2, "Wal2", cols=1)
            Wof_h = load_round(per, wo_f[0:128, :], [128, H], "Wof_h", raw_pool=raws)
            Wof_l = load_round(per, wo_f[128:F, :], [5, H], "Wof_l", raw_pool=raws)
            Wob = load_round(per, wo_b[:], [1, H], "Wob", raw_pool=raws)
            Wbb = load_round(per, wb_b[:], [1, H], "Wbb", raw_pool=raws)
            Ident = load_round(per, ident_in[:], [128, 128], "Ident", dt=f32)
            IdentB = load_round(per, ident_in[:], [128, 128], "IdentB", dt=bf16,
                                raw_pool=raws)
            Onesr = load_round(per, onesr_in[:], [1, 128], "Onesr", raw_pool=raws)
            OnesC = load_round(per, onesc_in[:], [128, 1], "OnesC", raw_pool=raws)
            Walb = load_round(per, walb[:], [1, 1], "Walb", dt=f32)
            OnesrF = load_round(per, onesr_in[:], [1, 128], "OnesrF", dt=f32)
            NbIdx = per.tile([128, BS // 128, 7], mybir.dt.int32, name="NbIdx")
            nc.sync.dma_start(NbIdx[:], nb_idx[:])
            RoIdx = per.tile([128, AS // 128, MNB], mybir.dt.int32, name="RoIdx")
            nc.sync.dma_start(RoIdx[:], ro_idx[:])


        # persistent activations
        InpT = per.tile([128, 2, BS], f32r, name="InpT")
        KT = per.tile([128, 2, B], bf16, name="KT")
        Vb = per.tile([128, B // 128, H + 1], bf16, name="Vb")
        QT = per.tile([128, 2, BS], bf16, name="QT")
        MsgTx = per.tile([128, 2, BS], f32r, name="MsgTx")   # local shard msgT
        KTl = per.tile([128, 2, BS], bf16, name="KTl")       # local K^T shard
        Vl = per.tile([128, BS // 128, H + 1], bf16, name="Vl")  # local V shard
        FeT_h = per.tile([128, AS], f32r, name="FeT_h")
        FeT_l = per.tile([5, AS], f32r, name="FeT_l")
        nc.vector.memset(Vl[:], 1.0)   # ones column persists across iters

        # packed per-core half-shard attention payload: [K^T 256x512 | V 512x257]
        HB = BS // 2
        KT_OFF = 0
        KT_SZ = H * HB                    # 131072
        V_OFF = KT_SZ
        V_SZ = HB * (H + 1)               # 131584
        PKT = V_OFF + V_SZ                # 262656
        pkt_in = [[dram.tile([PKT], bf16, name=f"pkt_in{t}_{h}")
                   for h in range(2)] for t in range(NITER)]
        pkt_out = [[dram.tile([NC * PKT], bf16, name=f"pkt_out{t}_{h}",
                              addr_space="Shared") for h in range(2)]
                   for t in range(NITER)]
        agn_ins = [dram.tile([BS, H], bf16, name=f"agn_in{t}")
                   for t in range(NITER + 1)]
        agn_outs = [dram.tile([B, H], bf16, name=f"agn_out{t}",
                              addr_space="Shared") for t in range(NITER + 1)]
        RG = [list(range(NC))]

        def emit_exchange_half(t, half, psum_pool, tag):
            """K^T/V for one bond-half of the shard, packed, one AllGather."""
            for ht in range(2):
                ps = psum_pool.tile([128, HB], f32, name=tag)
                for kt in range(2):
                    nc.tensor.matmul(ps[:], Wk[:, kt, bass.ts(ht, 128)],
                                     MsgTx[:, kt, bass.ts(half, HB)],
                                     start=(kt == 0), stop=(kt == 1))
                nc.vector.tensor_copy(KTl[:, ht, bass.ts(half, HB)], ps[:])
            nc.sync.dma_start(
                pkt_in[t][half][KT_OFF:KT_OFF + KT_SZ]
                .rearrange("(p t i) -> p t i", p=128, t=2),
                KTl[:, :, bass.ts(half, HB)])
            for cc in range(4):
                c = half * 4 + cc
                ps = psum_pool.tile([128, H], f32, name=tag)
                for kt in range(2):
                    nc.tensor.matmul(ps[:], MsgTx[:, kt, bass.ts(c, 128)],
                                     Wv[:, kt, :],
                                     start=(kt == 0), stop=(kt == 1))
                nc.vector.tensor_copy(Vl[:, c, 0:H], ps[:])
            nc.sync.dma_start(
                pkt_in[t][half][V_OFF:V_OFF + V_SZ]
                .rearrange("(c p h) -> p c h", p=128, c=4),
                Vl[:, bass.ts(half, 4), :])
            nc.gpsimd.collective_compute(
                "AllGather", ALU.bypass, RG,
                ins=[pkt_in[t][half].opt()], outs=[pkt_out[t][half].opt()])

        def emit_agn(dst_in, dst_out, psum_pool, tag, natc_pool):
            # natural layout via PE transposes, DMA per 128-bond block
            for c in range(BS // 128):
                psn = psum_pool.tile([128, H], f32, name=tag)
                for ht in range(2):
                    nc.tensor.transpose(
                        psn[:, bass.ts(ht, 128)],
                        MsgTx[:, ht, bass.ts(c, 128)].bitcast(f32), Ident[:])
                natc = natc_pool.tile([128, H], bf16, name="natc")
                nc.vector.tensor_copy(natc[:], psn[:])
                nc.sync.dma_start(dst_in[bass.ts(c, 128), :], natc[:])
            nc.gpsimd.collective_compute(
                "AllGather", ALU.bypass, RG,
                ins=[dst_in.opt()], outs=[dst_out.opt()])

        # ============ Phase B: message_0 = relu(f_bonds @ Wi) ==============
        with tc.tile_pool(name="pb", bufs=1) as pb, \
             tc.tile_pool(name="pb2", bufs=2) as pb2, \
             tc.tile_pool(name="pb_ps", bufs=4, space="PSUM") as pbps:
            FbT = load_round(pb, fbT[:].rearrange("(t p) i -> p t i", p=128),
                             [128, 2, BS], "FbT")
            for ht in range(2):
                for ch in range(BS // 512):
                    ps = pbps.tile([128, 512], f32, name="ps")
                    for kt in range(2):
                        nc.tensor.matmul(ps[:], Wi[:, kt, bass.ts(ht, 128)],
                                         FbT[:, kt, bass.ts(ch, 512)],
                                         start=(kt == 0), stop=(kt == 1))
                    nc.vector.tensor_copy(InpT[:, ht, bass.ts(ch, 512)], ps[:])
                    nc.scalar.activation(MsgTx[:, ht, bass.ts(ch, 512)], ps[:],
                                         AF.Relu)
            emit_exchange_half(0, 0, pbps, "ps")
            emit_exchange_half(0, 1, pbps, "ps")
            emit_agn(agn_ins[0], agn_outs[0], pbps, "ps", pb2)

        # ============ Phase A: per-molecule atom self-attention ============
        with tc.tile_pool(name="pa", bufs=1) as pa, \
             tc.tile_pool(name="pa2", bufs=2) as pa2, \
             tc.tile_pool(name="pa_ps", bufs=4, space="PSUM") as paps:
            WqA_h = load_round(pa, wq_a[0:128, :], [128, F], "WqA_h", dt=f32)
            WqA_l = load_round(pa, wq_a[128:F, :], [5, F], "WqA_l", dt=f32)
            WkA_h = load_round(pa, wk_a[0:128, :], [128, F], "WkA_h", dt=f32)
            WkA_l = load_round(pa, wk_a[128:F, :], [5, F], "WkA_l", dt=f32)
            WvA_h = load_round(pa, wv_a[0:128, :], [128, F], "WvA_h", dt=f32)
            WvA_l = load_round(pa, wv_a[128:F, :], [5, F], "WvA_l", dt=f32)
            LnG = load_round(pa, ln_g[:], [1, F], "LnG", dt=f32)
            LnB = load_round(pa, ln_b[:], [1, F], "LnB", dt=f32)
            FaT_h = load_round(pa, faT[0:128, :], [128, AS], "FaT_h", dt=f32)
            FaT_l = load_round(pa, faT[128:F, :], [5, AS], "FaT_l", dt=f32)
            XN = pa.tile([64, MPC, F], f32, name="XN")
            nc.sync.dma_start(XN[:], fa[:].rearrange("(m a) f -> a m f", a=64))
            GB = pa.tile([64, F], f32, name="GB")
            BB = pa.tile([64, F], f32, name="BB")
            for bc_src, bc_dst in ((LnG, GB), (LnB, BB)):
                ps = paps.tile([64, F], f32, name="ps")
                nc.tensor.matmul(ps[:], OnesrF[:, 0:64], bc_src[:],
                                 start=True, stop=True)
                nc.vector.tensor_copy(bc_dst[:], ps[:])

            def mm133(dst, lhs_pair, rhs_pair, n):
                ps = paps.tile([dst.shape[0], n], f32, name="ps")
                nc.tensor.matmul(ps[:], lhs_pair[0], rhs_pair[0],
                                 start=True, stop=False)
                nc.tensor.matmul(ps[:], lhs_pair[1], rhs_pair[1],
                                 start=False, stop=True)
                nc.vector.tensor_copy(dst, ps[:])

            QTa_h = pa.tile([128, AS], f32, name="QTa_h")
            QTa_l = pa.tile([5, AS], f32, name="QTa_l")
            KTa_h = pa.tile([128, AS], f32, name="KTa_h")
            KTa_l = pa.tile([5, AS], f32, name="KTa_l")
            mm133(QTa_h[:], (WqA_h[:, 0:128], WqA_l[:, 0:128]),
                  (FaT_h[:], FaT_l[:]), AS)
            mm133(QTa_l[:], (WqA_h[:, 128:F], WqA_l[:, 128:F]),
                  (FaT_h[:], FaT_l[:]), AS)
            mm133(KTa_h[:], (WkA_h[:, 0:128], WkA_l[:, 0:128]),
                  (FaT_h[:], FaT_l[:]), AS)
            mm133(KTa_l[:], (WkA_h[:, 128:F], WkA_l[:, 128:F]),
                  (FaT_h[:], FaT_l[:]), AS)
            VN = pa.tile([64, MPC, F], f32, name="VN")
            for m in range(MPC):
                mm133(VN[:, m, :],
                      (FaT_h[:, bass.ts(m, 64)], FaT_l[:, bass.ts(m, 64)]),
                      (WvA_h[:], WvA_l[:]), F)
            E = pa.tile([64, MPC, 64], f32, name="E")
            for m in range(MPC):
                ps = paps.tile([64, 64], f32, name="ps")
                nc.tensor.matmul(ps[:], QTa_h[:, bass.ts(m, 64)],
                                 KTa_h[:, bass.ts(m, 64)], start=True, stop=False)
                nc.tensor.matmul(ps[:], QTa_l[:, bass.ts(m, 64)],
                                 KTa_l[:, bass.ts(m, 64)], start=False, stop=True)
                nc.scalar.activation(E[:, m, :], ps[:], AF.Exp, scale=SCALE_ATOM)
            SumsA = pa.tile([64, MPC, 1], f32, name="SumsA")
            RS = pa.tile([64, MPC, 1], f32, name="RS")
            nc.vector.tensor_reduce(SumsA[:], E[:], axis=AX.X, op=ALU.add)
            nc.vector.reciprocal(RS[:], SumsA[:])
            AttnN = pa.tile([64, MPC, F], f32, name="AttnN")
            for m in range(MPC):
                pst = paps.tile([64, 64], f32, name="ps")
                nc.tensor.transpose(pst[:], E[:, m, :], Ident[0:64, 0:64])
                ET = pa2.tile([64, 64], f32, name="ET")
                nc.vector.tensor_copy(ET[:], pst[:])
                ps = paps.tile([64, F], f32, name="ps")
                nc.tensor.matmul(ps[:], ET[:], VN[:, m, :], start=True, stop=True)
                nc.vector.tensor_scalar_mul(AttnN[:, m, :], ps[:], RS[:, m, :])
            SumX = pa.tile([64, MPC, F], f32, name="SumX")
            nc.vector.tensor_tensor(SumX[:], XN[:], AttnN[:], op=ALU.add)
            Mu = pa.tile([64, MPC, 1], f32, name="Mu")
            nc.vector.tensor_reduce(Mu[:], SumX[:], axis=AX.X, op=ALU.add)
            nc.vector.tensor_scalar_mul(Mu[:], Mu[:], 1.0 / F)
            XC = pa.tile([64, MPC, F], f32, name="XC")
            for m in range(MPC):
                nc.vector.tensor_scalar_sub(XC[:, m, :], SumX[:, m, :], Mu[:, m, :])
            SQ = pa.tile([64, MPC, F], f32, name="SQ")
            nc.vector.tensor_tensor(SQ[:], XC[:], XC[:], op=ALU.mult)
            Var = pa.tile([64, MPC, 1], f32, name="Var")
            nc.vector.tensor_reduce(Var[:], SQ[:], axis=AX.X, op=ALU.add)
            Std = pa.tile([64, MPC, 1], f32, name="Std")
            EpsT = pa.tile([64, 1], f32, name="EpsT")
            nc.vector.memset(EpsT[:], 1e-5)
            nc.scalar.activation(Std[:], Var[:], AF.Sqrt, scale=1.0 / F,
                                 bias=EpsT[:])
            RStd = pa.tile([64, MPC, 1], f32, name="RStd")
            nc.vector.reciprocal(RStd[:], Std[:])
            FeN = pa.tile([64, MPC, F], f32, name="FeN")
            for m in range(MPC):
                nc.vector.tensor_scalar_mul(XC[:, m, :], XC[:, m, :], RStd[:, m, :])
                nc.vector.tensor_tensor(XC[:, m, :], XC[:, m, :], GB[:], op=ALU.mult)
                nc.vector.tensor_tensor(FeN[:, m, :], XC[:, m, :], BB[:], op=ALU.add)
            for m in range(MPC):
                ps1 = paps.tile([128, 64], f32, name="ps")
                nc.tensor.transpose(ps1[:], FeN[:, m, 0:128], Ident[0:64, 0:64])
                nc.vector.tensor_copy(FeT_h[:, bass.ts(m, 64)], ps1[:])
                ps2 = paps.tile([5, 64], f32, name="ps")
                nc.tensor.transpose(ps2[:], FeN[:, m, 128:F], Ident[0:64, 0:64])
                nc.vector.tensor_copy(FeT_l[:, bass.ts(m, 64)], ps2[:])

        # ============ Phase C: 3 message-passing iterations ================
        for it in range(NITER):
            mnall = agn_outs[it][:]
            with tc.tile_pool(name=f"pc{it}", bufs=1) as pc, \
                 tc.tile_pool(name=f"pm{it}", bufs=2) as pmt, \
                 tc.tile_pool(name=f"pcA{it}", bufs=2, space="PSUM") as ppsA, \
                 tc.tile_pool(name=f"pcS{it}", bufs=2, space="PSUM") as ppsS, \
                 tc.tile_pool(name=f"pcP{it}", bufs=1, space="PSUM") as ppsP:

                # ---- QT from the local shard (no collective dependency)
                for ch in range(2):
                    for ht in range(2):
                        ps = ppsA.tile([128, 512], f32, name="psA")
                        for kt in range(2):
                            nc.tensor.matmul(ps[:], Wq[:, kt, bass.ts(ht, 128)],
                                             MsgTx[:, kt, bass.ts(ch, 512)],
                                             start=(kt == 0), stop=(kt == 1))
                        nc.vector.tensor_copy(QT[:, ht, bass.ts(ch, 512)], ps[:])

                # ---- load gathered K^T / V shards from the half payloads
                for h in range(2):
                    for s in range(NC):
                        nc.sync.dma_start(
                            KT[:, :, s * BS + h * HB:s * BS + (h + 1) * HB],
                            pkt_out[it][h]
                            [s * PKT + KT_OFF:s * PKT + KT_OFF + KT_SZ]
                            .rearrange("(p t i) -> p t i", p=128, t=2))
                        nc.sync.dma_start(
                            Vb[:, s * 8 + h * 4:s * 8 + h * 4 + 4, :],
                            pkt_out[it][h]
                            [s * PKT + V_OFF:s * PKT + V_OFF + V_SZ]
                            .rearrange("(c p h) -> p c h", p=128, c=4))

                # ---- issue dmpnn gathers early (gpsimd) so they overlap the
                # ---- attention below (tensor engine); first-half bonds first
                Tn = pc.tile([128, BS // 128, H], bf16, name="Tn")
                for h in range(2):
                    for cc in range(4):
                        c = h * 4 + cc
                        nc.gpsimd.indirect_dma_start(
                            out=Tn[:, c, :], out_offset=None, in_=mnall,
                            in_offset=bass.IndirectOffsetOnAxis(
                                ap=NbIdx[:, c, 0:1], axis=0))
                    nc.vector.tensor_scalar_mul(Tn[:, bass.ts(h, 4), :],
                                                Tn[:, bass.ts(h, 4), :], -1.0)
                    for j in range(1, 7):
                        for cc in range(4):
                            c = h * 4 + cc
                            nc.gpsimd.indirect_dma_start(
                                out=Tn[:, c, :], out_offset=None, in_=mnall,
                                in_offset=bass.IndirectOffsetOnAxis(
                                    ap=NbIdx[:, c, j:j + 1], axis=0),
                                compute_op=ALU.add)

                # ---- attention (rows = shard), flash-style over j blocks,
                # ---- interleaved with the per-half tail so each half's
                # ---- exchange collective hides under the other half's rows
                patt = tc.alloc_tile_pool(name=f"patt{it}", bufs=1)
                AttA = patt.tile([128, BS // 128, H], f32r, name="AttA")
                AtT = patt.tile([128, 2, BS], f32r, name="AtT")
                AmT = patt.tile([128, 2, BS], f32r, name="AmT")
                TT = pc.tile([128, 2, BS], f32r, name="TT")
                DmT = pc.tile([128, 2, BS], f32r, name="DmT")
                Alp = pc.tile([1, BS], f32r, name="Alp")
                AlB = pc.tile([128, BS], f32, name="AlB")
                D1 = pc.tile([128, BS], f32, name="D1")

                def flash_rows(ic):
                    pvp = ppsP.tile([128, 4, 512], f32, name="psP")
                    prev = None
                    for jx in range(B // 128 + 1):
                        if jx < B // 128:
                            j = J_ORDER[jx]
                            sp = ppsS.tile([128, 512], f32, name="psS")
                            for kt in range(2):
                                nc.tensor.matmul(sp[:],
                                                 KT[:, kt, bass.ts(j, 128)],
                                                 QT[:, kt, bass.ts(ic, 512)],
                                                 start=(kt == 0),
                                                 stop=(kt == 1))
                            pt = pmt.tile([128, 512], bf16, name="PT")
                            nc.scalar.activation(pt[:], sp[:], AF.Exp,
                                                 scale=SCALE_BOND)
                        if prev is not None:
                            pj, ppt = prev
                            for isub in range(4):
                                nc.tensor.matmul(
                                    pvp[:, isub, 0:H + 1],
                                    ppt[:, bass.ts(isub, 128)],
                                    Vb[:, pj, 0:H + 1],
                                    start=(pj == J_ORDER[0]),
                                    stop=(pj == J_ORDER[-1]))
                        if jx < B // 128:
                            prev = (j, pt)
                    for isub in range(4):
                        rec = pmt.tile([128, 1], f32, name="rec")
                        nc.vector.reciprocal(rec[:], pvp[:, isub, H:H + 1])
                        nc.vector.tensor_scalar_mul(
                            AttA[:, ic * 4 + isub, :], pvp[:, isub, 0:H],
                            rec[:])

                def tail_half(h):
                    """att_msgT + dmpnn + alpha + combine for one bond half.

                    The dmpnn transposes read IdentF, an identity copy with an
                    artificial zero-valued dependency on this half's last
                    attention output, so the compile-time scheduler cannot
                    hoist this gather-dependent block into the attention
                    stream (that would stall the tensor engine on the slow
                    gathers)."""
                    hc = bass.ts(h, HB)
                    Zf = pmt.tile([128, 1], f32, name="Zf")
                    nc.vector.tensor_scalar_mul(
                        Zf[:], AttA[:, h * 4 + 3, 0:1].bitcast(f32), 0.0)
                    IdentF = pc.tile([128, 128], bf16, name=f"IdentF{h}")
                    nc.vector.tensor_scalar_add(IdentF[:], IdentB[:], Zf[:])
                    for ht in range(2):
                        pst = ppsA.tile([128, 512], f32, name="psA")
                        for cc in range(4):
                            c = h * 4 + cc
                            nc.tensor.transpose(
                                pst[:, bass.ts(cc, 128)],
                                AttA[:, c, bass.ts(ht, 128)].bitcast(f32),
                                Ident[:])
                        nc.vector.tensor_copy(AtT[:, ht, hc], pst[:])
                    for ht in range(2):
                        ps = ppsA.tile([128, 512], f32, name="psA")
                        for kt in range(2):
                            nc.tensor.matmul(ps[:], Wa[:, kt, bass.ts(ht, 128)],
                                             AtT[:, kt, hc],
                                             start=(kt == 0), stop=(kt == 1))
                        nc.vector.tensor_copy(AmT[:, ht, hc], ps[:])
                    for ht in range(2):
                        pst = ppsA.tile([128, 512], bf16, name="psA")
                        for cc in range(4):
                            c = h * 4 + cc
                            nc.tensor.transpose(
                                pst[:, bass.ts(cc, 128)],
                                Tn[:, c, bass.ts(ht, 128)],
                                IdentF[:])
                        nc.vector.tensor_copy(TT[:, ht, hc], pst[:])
                    for ht in range(2):
                        ps = ppsA.tile([128, 512], f32, name="psA")
                        for kt in range(2):
                            nc.tensor.matmul(ps[:], Wh[:, kt, bass.ts(ht, 128)],
                                             TT[:, kt, hc],
                                             start=(kt == 0), stop=(kt == 1))
                        nc.vector.tensor_copy(DmT[:, ht, hc], ps[:])
                    ps = ppsA.tile([1, 512], f32, name="psA")
                    nc.tensor.matmul(ps[:], Wal1[:, 0, :], DmT[:, 0, hc],
                                     start=True, stop=False)
                    nc.tensor.matmul(ps[:], Wal1[:, 1, :], DmT[:, 1, hc],
                                     start=False, stop=False)
                    nc.tensor.matmul(ps[:], Wal2[:, 0, :], AmT[:, 0, hc],
                                     start=False, stop=False)
                    nc.tensor.matmul(ps[:], Wal2[:, 1, :], AmT[:, 1, hc],
                                     start=False, stop=True)
                    nc.scalar.activation(Alp[:, hc], ps[:], AF.Sigmoid,
                                         bias=Walb[:])
                    ps = ppsA.tile([128, 512], f32, name="psA")
                    nc.tensor.matmul(ps[:], Onesr[:], Alp[:, hc],
                                     start=True, stop=True)
                    nc.vector.tensor_copy(AlB[:, hc], ps[:])
                    for ht in range(2):
                        d1 = D1[:, hc]
                        nc.vector.tensor_tensor(d1, DmT[:, ht, hc],
                                                AmT[:, ht, hc],
                                                op=ALU.subtract)
                        nc.vector.tensor_tensor(d1, d1, AlB[:, hc],
                                                op=ALU.mult)
                        nc.vector.tensor_tensor(d1, d1, AmT[:, ht, hc],
                                                op=ALU.add)
                        nc.vector.tensor_tensor(d1, d1, InpT[:, ht, hc],
                                                op=ALU.add)
                        nc.scalar.activation(MsgTx[:, ht, hc], d1, AF.Relu)

                for h in range(2):
                    flash_rows(h)
                    tail_half(h)
                    if it + 1 < NITER:
                        emit_exchange_half(it + 1, h, ppsA, "psA")
                patt.release()
                emit_agn(agn_ins[it + 1], agn_outs[it + 1], ppsA, "psA", pmt)

        # ============ Readout + per-molecule pooling =======================
        with tc.tile_pool(name="pd", bufs=1) as pd, \
             tc.tile_pool(name="pd2", bufs=2) as pd2, \
             tc.tile_pool(name="pd_ps", bufs=4, space="PSUM") as pdps:
            AmN = pd.tile([128, AS // 128, H], bf16, name="AmN")
            for c in range(AS // 128):
                nc.gpsimd.indirect_dma_start(
                    out=AmN[:, c, :], out_offset=None, in_=agn_outs[NITER][:],
                    in_offset=bass.IndirectOffsetOnAxis(
                        ap=RoIdx[:, c, 0:1], axis=0))
            for j in range(1, MNB):
                for c in range(AS // 128):
                    nc.gpsimd.indirect_dma_start(
                        out=AmN[:, c, :], out_offset=None, in_=agn_outs[NITER][:],
                        in_offset=bass.IndirectOffsetOnAxis(
                            ap=RoIdx[:, c, j:j + 1], axis=0),
                        compute_op=ALU.add)
            AmT2 = pd.tile([128, 2, AS], f32r, name="AmT2")
            for ht in range(2):
                pst = pdps.tile([128, AS], bf16, name="ps")
                for c in range(AS // 128):
                    nc.tensor.transpose(pst[:, bass.ts(c, 128)],
                                        AmN[:, c, bass.ts(ht, 128)],
                                        IdentB[:])
                nc.vector.tensor_copy(AmT2[:, ht, :], pst[:])
            Hm = pd.tile([128, AS // 128, H], f32r, name="Hm")
            for c in range(AS // 128):
                ps = pdps.tile([128, H], f32, name="ps")
                nc.tensor.matmul(ps[:], FeT_h[:, bass.ts(c, 128)], Wof_h[:],
                                 start=True, stop=False)
                nc.tensor.matmul(ps[:], FeT_l[:, bass.ts(c, 128)], Wof_l[:],
                                 start=False, stop=False)
                for kt in range(2):
                    nc.tensor.matmul(ps[:], AmT2[:, kt, bass.ts(c, 128)],
                                     Wom[:, kt, :], start=False, stop=False)
                nc.tensor.matmul(ps[:], Onesr[:, 0:128], Wob[:],
                                 start=False, stop=True)
                nc.scalar.activation(Hm[:, c, :], ps[:], AF.Relu)
            HmT = pd.tile([128, 2, AS], f32r, name="HmT")
            for ht in range(2):
                pst = pdps.tile([128, AS], f32, name="ps")
                for c in range(AS // 128):
                    nc.tensor.transpose(pst[:, bass.ts(c, 128)],
                                        Hm[:, c, bass.ts(ht, 128)].bitcast(f32),
                                        Ident[:])
                nc.vector.tensor_copy(HmT[:, ht, :], pst[:])
            # hm in per-molecule base-0 layout via SBUF->SBUF DMA
            HmM = pd.tile([64, MPC, H], f32r, name="HmM")
            nc.sync.dma_start(HmM[:, 0:MPC:2, :], Hm[0:64, :, :])
            nc.sync.dma_start(HmM[:, 1:MPC:2, :], Hm[64:128, :, :])
            T2T = pd.tile([128, 2, AS], f32r, name="T2T")
            for ht in range(2):
                ps = pdps.tile([128, AS], f32, name="ps")
                for kt in range(2):
                    nc.tensor.matmul(ps[:], Wa[:, kt, bass.ts(ht, 128)],
                                     HmT[:, kt, :], start=(kt == 0), stop=(kt == 1))
                nc.vector.tensor_copy(T2T[:, ht, :], ps[:])
            SC2 = pd.tile([64, MPC, 64], f32, name="SC2")
            for m in range(MPC):
                ps = pdps.tile([64, 64], f32, name="ps")
                for kt in range(2):
                    nc.tensor.matmul(ps[:], T2T[:, kt, bass.ts(m, 64)],
                                     HmT[:, kt, bass.ts(m, 64)],
                                     start=(kt == 0), stop=(kt == 1))
                nc.vector.tensor_copy(SC2[:, m, :], ps[:])
            Mx2 = pd.tile([64, MPC, 1], f32, name="Mx2")
            nc.vector.tensor_reduce(Mx2[:], SC2[:], axis=AX.X, op=ALU.max)
            NMx2 = pd.tile([64, MPC, 1], f32, name="NMx2")
            nc.vector.tensor_scalar_mul(NMx2[:], Mx2[:], -1.0)
            E2 = pd.tile([64, MPC, 64], f32, name="E2")
            for m in range(MPC):
                nc.scalar.activation(E2[:, m, :], SC2[:, m, :], AF.Exp,
                                     bias=NMx2[:, m, :])
            Sum2 = pd.tile([64, MPC, 1], f32, name="Sum2")
            RS2 = pd.tile([64, MPC, 1], f32, name="RS2")
            nc.vector.tensor_reduce(Sum2[:], E2[:], axis=AX.X, op=ALU.add)
            nc.vector.reciprocal(RS2[:], Sum2[:])
            BB2 = pd.tile([64, H], f32, name="BB2")
            psbb = pdps.tile([64, H], f32, name="ps")
            nc.tensor.matmul(psbb[:], Onesr[:, 0:64], Wbb[:], start=True, stop=True)
            nc.vector.tensor_copy(BB2[:], psbb[:])
            OutS = pd.tile([1, MPC, H], f32, name="OutS")
            for m in range(MPC):
                pst = pdps.tile([64, 64], f32, name="ps")
                nc.tensor.transpose(pst[:], E2[:, m, :], Ident[0:64, 0:64])
                E2T = pd2.tile([64, 64], f32r, name="E2T")
                nc.vector.tensor_copy(E2T[:], pst[:])
                UT = pd2.tile([128, 2, 64], f32r, name="UT")
                for hs in range(2):
                    psu = pdps.tile([128, 64], f32, name="ps")
                    nc.tensor.matmul(psu[:], HmM[:, m, bass.ts(hs, 128)], E2T[:],
                                     start=True, stop=True)
                    nc.vector.tensor_copy(UT[:, hs, :], psu[:])
                psb = pdps.tile([64, H], f32, name="ps")
                for kt in range(2):
                    nc.tensor.matmul(psb[:], UT[:, kt, :], Wbw[:, kt, :],
                                     start=(kt == 0), stop=(kt == 1))
                AH = pd2.tile([64, H], f32, name="AH")
                nc.vector.tensor_scalar_mul(AH[:], psb[:], RS2[:, m, :])
                nc.vector.tensor_tensor(AH[:], AH[:], BB2[:], op=ALU.add)
                nc.vector.tensor_scalar_max(AH[:], AH[:], 0.0)
                XS = pd2.tile([64, H], f32r, name="XS")
                nc.vector.tensor_tensor(XS[:], AH[:], HmM[:, m, :], op=ALU.add)
                psm = pdps.tile([1, H], f32, name="ps")
                nc.tensor.matmul(psm[:], OnesC[0:64, :], XS[:],
                                 start=True, stop=True)
                nc.vector.tensor_scalar_mul(OutS[:, m, :], psm[:], 1.0 / APM)
            nc.sync.dma_start(out_mol[:].rearrange("(o m) h -> o m h", o=1), OutS[:])

    nc.compile()
    return nc


def _host_prepare(inputs):
    f_atoms = np.asarray(inputs["f_atoms"], np.float32)
    f_bonds = np.asarray(inputs["f_bonds"], np.float32)
    a2b = np.asarray(inputs["a2b"]).astype(np.int64)
    b2a = np.asarray(inputs["b2a"]).astype(np.int64)
    b2revb = np.asarray(inputs["b2revb"]).astype(np.int64)

    fbp = np.zeros((B, H), np.float32)
    fbp[:, :FD] = f_bonds
    fbT_full = np.ascontiguousarray(fbp.T)
    faT_full = np.ascontiguousarray(f_atoms.T)

    W = {k: np.asarray(inputs[k], np.float32) for k in
         ("Wq_atom", "Wk_atom", "Wv_atom", "Wi", "Wh", "Wq", "Wk", "Wv", "Wa",
          "Walpha_w", "Wo_w", "Wb_w")}
    wi_p = np.zeros((H, H), np.float32)
    wi_p[:FD, :] = W["Wi"]

    base = dict(
        wq_a=W["Wq_atom"], wk_a=W["Wk_atom"], wv_a=W["Wv_atom"],
        ln_g=np.asarray(inputs["ln_g"], np.float32).reshape(1, F),
        ln_b=np.asarray(inputs["ln_b"], np.float32).reshape(1, F),
        wi=wi_p, wh=W["Wh"], wq=W["Wq"], wk=W["Wk"], wv=W["Wv"], wa=W["Wa"],
        wal1=np.ascontiguousarray(W["Walpha_w"][:H]),
        wal2=np.ascontiguousarray(W["Walpha_w"][H:]),
        walb=np.asarray(inputs["Walpha_b"], np.float32).reshape(1, 1),
        wo_f=np.ascontiguousarray(W["Wo_w"][:F]),
        wo_m=np.ascontiguousarray(W["Wo_w"][F:]),
        wo_b=np.asarray(inputs["Wo_b"], np.float32).reshape(1, H),
        wb_w=W["Wb_w"],
        wb_b=np.asarray(inputs["Wb_b"], np.float32).reshape(1, H),
        ident=np.eye(128, dtype=np.float32),
        onesr=np.ones((1, 128), np.float32),
        onesc=np.ones((128, 1), np.float32),
    )

    in_maps = []
    for c in range(NC):
        bonds = np.arange(c * BS, (c + 1) * BS)
        # [7, BS]: row 0 = b2revb (subtracted), rows 1..6 = a2b[b2a]
        terms = np.stack([b2revb[bonds]]
                         + [a2b[b2a[bonds], j] for j in range(MNB)])
        nb2 = np.ascontiguousarray(
            terms.reshape(7, BS // 128, 128).transpose(2, 1, 0)).astype(np.int32)
        atoms = np.arange(c * AS, (c + 1) * AS)
        ro = np.stack([a2b[atoms, j] for j in range(MNB)])  # [6, AS]
        ro2 = np.ascontiguousarray(
            ro.reshape(MNB, AS // 128, 128).transpose(2, 1, 0)).astype(np.int32)
        m = dict(base)
        m["fbT"] = np.ascontiguousarray(fbT_full[:, bonds])
        m["faT"] = np.ascontiguousarray(faT_full[:, atoms])
        m["fa"] = np.ascontiguousarray(f_atoms[atoms])
        m["nb_idx"] = nb2
        m["ro_idx"] = ro2
        in_maps.append(m)
    return in_maps


def kernel(**inputs):
    if "nc" not in _CACHE:
        _CACHE["nc"] = _build()
    nc = _CACHE["nc"]
    in_maps = _host_prepare(inputs)
    res = run_bass_kernel_spmd(nc, in_maps, core_ids=list(range(NC)))
    out = np.concatenate([res.results[c]["mol_out"] for c in range(NC)], 0)
    return np.ascontiguousarray(out.astype(np.float32))


# revision 60
# speedup vs baseline: 1.0515x; 1.0315x over previous
"""Trainium2 Bass kernel for nn_HGNNEncoder (DMPNN + global bond attention).

Sharding: data-parallel over bonds/atoms/molecules across 8 NeuronCores.
The global bond-bond attention stays exact: each core computes its 1024-row
slice of the 8192x8192 attention (bf16 QK^T, exp on ScalarE with the 1/16
scale folded in, bf16 P@[V|1] with fused row-sum column, normalized
post-hoc).  Between iterations each core computes its K^T / V column shard
locally and the shards are exchanged with half-sized bf16 AllGathers (so
attention on first-half columns starts while the second half is in flight);
the natural-layout message is AllGathered in bf16 for the DMPNN row-gathers,
which run on GpSimd fully overlapped with the attention.
"""

import numpy as np

import concourse.bass as bass
import concourse.bacc as bacc
import concourse.mybir as mybir
import concourse.tile as tile
from concourse.bass_utils import run_bass_kernel_spmd

NC = 8
B, NA, MNB = 8192, 4096, 6
H = 256
F = 133
FD = 147
BS = B // NC          # 1024 bonds per core
AS = NA // NC         # 512 atoms per core
APM = 64              # atoms per molecule
MPC = AS // APM       # 8 molecules per core
NITER = 3

f32 = mybir.dt.float32
f32r = mybir.dt.float32r
bf16 = mybir.dt.bfloat16
AF = mybir.ActivationFunctionType
ALU = mybir.AluOpType
AX = mybir.AxisListType

SCALE_BOND = float(1.0 / np.sqrt(np.float32(H)))
SCALE_ATOM = float(1.0 / np.sqrt(np.float32(F)))

# flash-loop column-block order: every core exchanges the first half of its
# bond shard first, so all first-half column blocks (j%8 < 4) arrive first
J_ORDER = [j for j in range(B // 128) if j % 8 < 4] + \
          [j for j in range(B // 128) if j % 8 >= 4]

_CACHE = {}


def _build():
    nc = bacc.Bacc("TRN2", target_bir_lowering=False, debug=False, num_devices=NC)

    def inp(name, shape, dt=f32):
        return nc.dram_tensor(name, list(shape), dt, kind="ExternalInput")

    fbT = inp("fbT", [H, BS])
    faT = inp("faT", [F, AS])
    fa = inp("fa", [AS, F])
    wq_a = inp("wq_a", [F, F]); wk_a = inp("wk_a", [F, F]); wv_a = inp("wv_a", [F, F])
    ln_g = inp("ln_g", [1, F]); ln_b = inp("ln_b", [1, F])
    wi = inp("wi", [H, H])
    wh = inp("wh", [H, H]); wq = inp("wq", [H, H]); wk = inp("wk", [H, H])
    wv = inp("wv", [H, H]); wa = inp("wa", [H, H])
    wal1 = inp("wal1", [H, 1]); wal2 = inp("wal2", [H, 1]); walb = inp("walb", [1, 1])
    wo_f = inp("wo_f", [F, H]); wo_m = inp("wo_m", [H, H]); wo_b = inp("wo_b", [1, H])
    wb_w = inp("wb_w", [H, H]); wb_b = inp("wb_b", [1, H])
    ident_in = inp("ident", [128, 128])
    onesr_in = inp("onesr", [1, 128])
    onesc_in = inp("onesc", [128, 1])
    nb_idx = inp("nb_idx", [128, BS // 128, 7], mybir.dt.int32)
    ro_idx = inp("ro_idx", [128, AS // 128, MNB], mybir.dt.int32)
    out_mol = nc.dram_tensor("mol_out", [MPC, H], f32, kind="ExternalOutput")

    with tile.TileContext(nc) as tc, \
         tc.tile_pool(name="persist", bufs=1) as per, \
         tc.tile_pool(name="dram", bufs=1, space="DRAM") as dram:

        def load_round(pool, src_ap, shape, name, dt=f32r, raw_pool=None):
            raw = (raw_pool or pool).tile(list(shape), f32, name=name + "_raw")
            nc.sync.dma_start(raw[:], src_ap)
            if dt == f32:
                return raw
            t = pool.tile(list(shape), dt, name=name)
            nc.vector.tensor_copy(t[:], raw[:])
            return t

        # ---------------- persistent weights ----------------
        with tc.tile_pool(name="raws", bufs=1) as raws:
            def loadw(src, name, cols=H):
                return load_round(per, src[:].rearrange("(t p) h -> p t h", p=128),
                                  [128, 2, cols], name, raw_pool=raws)
            Wi = loadw(wi, "Wi"); Wh = loadw(wh, "Wh"); Wq = loadw(wq, "Wq")
            Wk = loadw(wk, "Wk"); Wv = loadw(wv, "Wv"); Wa = loadw(wa, "Wa")
            Wom = loadw(wo_m, "Wom"); Wbw = loadw(wb_w, "Wbw")
            Wal1 = loadw(wal1, "Wal1", cols=1); Wal2 = loadw(wal2, "Wal2", cols=1)
            Wof_h = load_round(per, wo_f[0:128, :], [128, H], "Wof_h", raw_pool=raws)
            Wof_l = load_round(per, wo_f[128:F, :], [5, H], "Wof_l", raw_pool=raws)
            Wob = load_round(per, wo_b[:], [1, H], "Wob", raw_pool=raws)
            Wbb = load_round(per, wb_b[:], [1, H], "Wbb", raw_pool=raws)
            Ident = load_round(per, ident_in[:], [128, 128], "Ident", dt=f32)
            IdentB = load_round(per, ident_in[:], [128, 128], "IdentB", dt=bf16,
                                raw_pool=raws)
            Onesr = load_round(per, onesr_in[:], [1, 128], "Onesr", raw_pool=raws)
            OnesC = load_round(per, onesc_in[:], [128, 1], "OnesC", raw_pool=raws)
            Walb = load_round(per, walb[:], [1, 1], "Walb", dt=f32)
            OnesrF = load_round(per, onesr_in[:], [1, 128], "OnesrF", dt=f32)
            NbIdx = per.tile([128, BS // 128, 7], mybir.dt.int32, name="NbIdx")
            nc.sync.dma_start(NbIdx[:], nb_idx[:])
            RoIdx = per.tile([128, AS // 128, MNB], mybir.dt.int32, name="RoIdx")
            nc.sync.dma_start(RoIdx[:], ro_idx[:])


        # persistent activations
        InpT = per.tile([128, 2, BS], f32r, name="InpT")
        KT = per.tile([128, 2, B], bf16, name="KT")
        Vb = per.tile([128, B // 128, H + 1], bf16, name="Vb")
        QT = per.tile([128, 2, BS], bf16, name="QT")
        MsgTx = per.tile([128, 2, BS], f32r, name="MsgTx")   # local shard msgT
        KTl = per.tile([128, 2, BS], bf16, name="KTl")       # local K^T shard
        Vl = per.tile([128, BS // 128, H + 1], bf16, name="Vl")  # local V shard
        FeT_h = per.tile([128, AS], f32r, name="FeT_h")
        FeT_l = per.tile([5, AS], f32r, name="FeT_l")
        nc.vector.memset(Vl[:], 1.0)   # ones column persists across iters

        # packed per-core half-shard attention payload: [K^T 256x512 | V 512x257]
        HB = BS // 2
        KT_OFF = 0
        KT_SZ = H * HB                    # 131072
        V_OFF = KT_SZ
        V_SZ = HB * (H + 1)               # 131584
        PKT = V_OFF + V_SZ                # 262656
        pkt_in = [[dram.tile([PKT], bf16, name=f"pkt_in{t}_{h}")
                   for h in range(2)] for t in range(NITER)]
        pkt_out = [[dram.tile([NC * PKT], bf16, name=f"pkt_out{t}_{h}",
                              addr_space="Shared") for h in range(2)]
                   for t in range(NITER)]
        agn_ins = [dram.tile([BS, H], bf16, name=f"agn_in{t}")
                   for t in range(NITER + 1)]
        agn_outs = [dram.tile([B, H], bf16, name=f"agn_out{t}",
                              addr_space="Shared") for t in range(NITER + 1)]
        RG = [list(range(NC))]

        def emit_exchange_half(t, half, psum_pool, tag):
            """K^T/V for one bond-half of the shard, packed, one AllGather."""
            for ht in range(2):
                ps = psum_pool.tile([128, HB], f32, name=tag)
                for kt in range(2):
                    nc.tensor.matmul(ps[:], Wk[:, kt, bass.ts(ht, 128)],
                                     MsgTx[:, kt, bass.ts(half, HB)],
                                     start=(kt == 0), stop=(kt == 1))
                nc.vector.tensor_copy(KTl[:, ht, bass.ts(half, HB)], ps[:])
            nc.sync.dma_start(
                pkt_in[t][half][KT_OFF:KT_OFF + KT_SZ]
                .rearrange("(p t i) -> p t i", p=128, t=2),
                KTl[:, :, bass.ts(half, HB)])
            for cc in range(4):
                c = half * 4 + cc
                ps = psum_pool.tile([128, H], f32, name=tag)
                for kt in range(2):
                    nc.tensor.matmul(ps[:], MsgTx[:, kt, bass.ts(c, 128)],
                                     Wv[:, kt, :],
                                     start=(kt == 0), stop=(kt == 1))
                nc.vector.tensor_copy(Vl[:, c, 0:H], ps[:])
            nc.sync.dma_start(
                pkt_in[t][half][V_OFF:V_OFF + V_SZ]
                .rearrange("(c p h) -> p c h", p=128, c=4),
                Vl[:, bass.ts(half, 4), :])
            nc.gpsimd.collective_compute(
                "AllGather", ALU.bypass, RG,
                ins=[pkt_in[t][half].opt()], outs=[pkt_out[t][half].opt()])

        def emit_agn(dst_in, dst_out, psum_pool, tag, natc_pool):
            # natural layout via PE transposes, DMA per 128-bond block
            for c in range(BS // 128):
                psn = psum_pool.tile([128, H], f32, name=tag)
                for ht in range(2):
                    nc.tensor.transpose(
                        psn[:, bass.ts(ht, 128)],
                        MsgTx[:, ht, bass.ts(c, 128)].bitcast(f32), Ident[:])
                natc = natc_pool.tile([128, H], bf16, name="natc")
                nc.vector.tensor_copy(natc[:], psn[:])
                nc.sync.dma_start(dst_in[bass.ts(c, 128), :], natc[:])
            nc.gpsimd.collective_compute(
                "AllGather", ALU.bypass, RG,
                ins=[dst_in.opt()], outs=[dst_out.opt()])

        # ============ Phase B: message_0 = relu(f_bonds @ Wi) ==============
        with tc.tile_pool(name="pb", bufs=1) as pb, \
             tc.tile_pool(name="pb2", bufs=2) as pb2, \
             tc.tile_pool(name="pb_ps", bufs=4, space="PSUM") as pbps:
            FbT = load_round(pb, fbT[:].rearrange("(t p) i -> p t i", p=128),
                             [128, 2, BS], "FbT")
            for ht in range(2):
                for ch in range(BS // 512):
                    ps = pbps.tile([128, 512], f32, name="ps")
                    for kt in range(2):
                        nc.tensor.matmul(ps[:], Wi[:, kt, bass.ts(ht, 128)],
                                         FbT[:, kt, bass.ts(ch, 512)],
                                         start=(kt == 0), stop=(kt == 1))
                    nc.vector.tensor_copy(InpT[:, ht, bass.ts(ch, 512)], ps[:])
                    nc.scalar.activation(MsgTx[:, ht, bass.ts(ch, 512)], ps[:],
                                         AF.Relu)
            # agn first: iteration 0's gathers depend on it and start ~60us
            # earlier this way, hiding under iteration 0's attention
            emit_agn(agn_ins[0], agn_outs[0], pbps, "ps", pb2)
            emit_exchange_half(0, 0, pbps, "ps")
            emit_exchange_half(0, 1, pbps, "ps")

        # ============ Phase A: per-molecule atom self-attention ============
        with tc.tile_pool(name="pa", bufs=1) as pa, \
             tc.tile_pool(name="pa2", bufs=2) as pa2, \
             tc.tile_pool(name="pa_ps", bufs=4, space="PSUM") as paps:
            WqA_h = load_round(pa, wq_a[0:128, :], [128, F], "WqA_h", dt=f32)
            WqA_l = load_round(pa, wq_a[128:F, :], [5, F], "WqA_l", dt=f32)
            WkA_h = load_round(pa, wk_a[0:128, :], [128, F], "WkA_h", dt=f32)
            WkA_l = load_round(pa, wk_a[128:F, :], [5, F], "WkA_l", dt=f32)
            WvA_h = load_round(pa, wv_a[0:128, :], [128, F], "WvA_h", dt=f32)
            WvA_l = load_round(pa, wv_a[128:F, :], [5, F], "WvA_l", dt=f32)
            LnG = load_round(pa, ln_g[:], [1, F], "LnG", dt=f32)
            LnB = load_round(pa, ln_b[:], [1, F], "LnB", dt=f32)
            FaT_h = load_round(pa, faT[0:128, :], [128, AS], "FaT_h", dt=f32)
            FaT_l = load_round(pa, faT[128:F, :], [5, AS], "FaT_l", dt=f32)
            XN = pa.tile([64, MPC, F], f32, name="XN")
            nc.sync.dma_start(XN[:], fa[:].rearrange("(m a) f -> a m f", a=64))
            GB = pa.tile([64, F], f32, name="GB")
            BB = pa.tile([64, F], f32, name="BB")
            for bc_src, bc_dst in ((LnG, GB), (LnB, BB)):
                ps = paps.tile([64, F], f32, name="ps")
                nc.tensor.matmul(ps[:], OnesrF[:, 0:64], bc_src[:],
                                 start=True, stop=True)
                nc.vector.tensor_copy(bc_dst[:], ps[:])

            def mm133(dst, lhs_pair, rhs_pair, n):
                ps = paps.tile([dst.shape[0], n], f32, name="ps")
                nc.tensor.matmul(ps[:], lhs_pair[0], rhs_pair[0],
                                 start=True, stop=False)
                nc.tensor.matmul(ps[:], lhs_pair[1], rhs_pair[1],
                                 start=False, stop=True)
                nc.vector.tensor_copy(dst, ps[:])

            QTa_h = pa.tile([128, AS], f32, name="QTa_h")
            QTa_l = pa.tile([5, AS], f32, name="QTa_l")
            KTa_h = pa.tile([128, AS], f32, name="KTa_h")
            KTa_l = pa.tile([5, AS], f32, name="KTa_l")
            mm133(QTa_h[:], (WqA_h[:, 0:128], WqA_l[:, 0:128]),
                  (FaT_h[:], FaT_l[:]), AS)
            mm133(QTa_l[:], (WqA_h[:, 128:F], WqA_l[:, 128:F]),
                  (FaT_h[:], FaT_l[:]), AS)
            mm133(KTa_h[:], (WkA_h[:, 0:128], WkA_l[:, 0:128]),
                  (FaT_h[:], FaT_l[:]), AS)
            mm133(KTa_l[:], (WkA_h[:, 128:F], WkA_l[:, 128:F]),
                  (FaT_h[:], FaT_l[:]), AS)
            VN = pa.tile([64, MPC, F], f32, name="VN")
            for m in range(MPC):
                mm133(VN[:, m, :],
                      (FaT_h[:, bass.ts(m, 64)], FaT_l[:, bass.ts(m, 64)]),
                      (WvA_h[:], WvA_l[:]), F)
            E = pa.tile([64, MPC, 64], f32, name="E")
            for m in range(MPC):
                ps = paps.tile([64, 64], f32, name="ps")
                nc.tensor.matmul(ps[:], QTa_h[:, bass.ts(m, 64)],
                                 KTa_h[:, bass.ts(m, 64)], start=True, stop=False)
                nc.tensor.matmul(ps[:], QTa_l[:, bass.ts(m, 64)],
                                 KTa_l[:, bass.ts(m, 64)], start=False, stop=True)
                nc.scalar.activation(E[:, m, :], ps[:], AF.Exp, scale=SCALE_ATOM)
            SumsA = pa.tile([64, MPC, 1], f32, name="SumsA")
            RS = pa.tile([64, MPC, 1], f32, name="RS")
            nc.vector.tensor_reduce(SumsA[:], E[:], axis=AX.X, op=ALU.add)
            nc.vector.reciprocal(RS[:], SumsA[:])
            AttnN = pa.tile([64, MPC, F], f32, name="AttnN")
            for m in range(MPC):
                pst = paps.tile([64, 64], f32, name="ps")
                nc.tensor.transpose(pst[:], E[:, m, :], Ident[0:64, 0:64])
                ET = pa2.tile([64, 64], f32, name="ET")
                nc.vector.tensor_copy(ET[:], pst[:])
                ps = paps.tile([64, F], f32, name="ps")
                nc.tensor.matmul(ps[:], ET[:], VN[:, m, :], start=True, stop=True)
                nc.vector.tensor_scalar_mul(AttnN[:, m, :], ps[:], RS[:, m, :])
            SumX = pa.tile([64, MPC, F], f32, name="SumX")
            nc.vector.tensor_tensor(SumX[:], XN[:], AttnN[:], op=ALU.add)
            Mu = pa.tile([64, MPC, 1], f32, name="Mu")
            nc.vector.tensor_reduce(Mu[:], SumX[:], axis=AX.X, op=ALU.add)
            nc.vector.tensor_scalar_mul(Mu[:], Mu[:], 1.0 / F)
            XC = pa.tile([64, MPC, F], f32, name="XC")
            for m in range(MPC):
                nc.vector.tensor_scalar_sub(XC[:, m, :], SumX[:, m, :], Mu[:, m, :])
            SQ = pa.tile([64, MPC, F], f32, name="SQ")
            nc.vector.tensor_tensor(SQ[:], XC[:], XC[:], op=ALU.mult)
            Var = pa.tile([64, MPC, 1], f32, name="Var")
            nc.vector.tensor_reduce(Var[:], SQ[:], axis=AX.X, op=ALU.add)
            Std = pa.tile([64, MPC, 1], f32, name="Std")
            EpsT = pa.tile([64, 1], f32, name="EpsT")
            nc.vector.memset(EpsT[:], 1e-5)
            nc.scalar.activation(Std[:], Var[:], AF.Sqrt, scale=1.0 / F,
                                 bias=EpsT[:])
            RStd = pa.tile([64, MPC, 1], f32, name="RStd")
            nc.vector.reciprocal(RStd[:], Std[:])
            FeN = pa.tile([64, MPC, F], f32, name="FeN")
            for m in range(MPC):
                nc.vector.tensor_scalar_mul(XC[:, m, :], XC[:, m, :], RStd[:, m, :])
                nc.vector.tensor_tensor(XC[:, m, :], XC[:, m, :], GB[:], op=ALU.mult)
                nc.vector.tensor_tensor(FeN[:, m, :], XC[:, m, :], BB[:], op=ALU.add)
            for m in range(MPC):
                ps1 = paps.tile([128, 64], f32, name="ps")
                nc.tensor.transpose(ps1[:], FeN[:, m, 0:128], Ident[0:64, 0:64])
                nc.vector.tensor_copy(FeT_h[:, bass.ts(m, 64)], ps1[:])
                ps2 = paps.tile([5, 64], f32, name="ps")
                nc.tensor.transpose(ps2[:], FeN[:, m, 128:F], Ident[0:64, 0:64])
                nc.vector.tensor_copy(FeT_l[:, bass.ts(m, 64)], ps2[:])

        # ============ Phase C: 3 message-passing iterations ================
        for it in range(NITER):
            mnall = agn_outs[it][:]
            with tc.tile_pool(name=f"pc{it}", bufs=1) as pc, \
                 tc.tile_pool(name=f"pm{it}", bufs=2) as pmt, \
                 tc.tile_pool(name=f"pcA{it}", bufs=2, space="PSUM") as ppsA, \
                 tc.tile_pool(name=f"pcS{it}", bufs=2, space="PSUM") as ppsS, \
                 tc.tile_pool(name=f"pcP{it}", bufs=1, space="PSUM") as ppsP:

                # ---- QT from the local shard (no collective dependency)
                for ch in range(2):
                    for ht in range(2):
                        ps = ppsA.tile([128, 512], f32, name="psA")
                        for kt in range(2):
                            nc.tensor.matmul(ps[:], Wq[:, kt, bass.ts(ht, 128)],
                                             MsgTx[:, kt, bass.ts(ch, 512)],
                                             start=(kt == 0), stop=(kt == 1))
                        nc.vector.tensor_copy(QT[:, ht, bass.ts(ch, 512)], ps[:])

                # ---- load gathered K^T / V shards from the half payloads
                for h in range(2):
                    for s in range(NC):
                        nc.sync.dma_start(
                            KT[:, :, s * BS + h * HB:s * BS + (h + 1) * HB],
                            pkt_out[it][h]
                            [s * PKT + KT_OFF:s * PKT + KT_OFF + KT_SZ]
                            .rearrange("(p t i) -> p t i", p=128, t=2))
                        nc.sync.dma_start(
                            Vb[:, s * 8 + h * 4:s * 8 + h * 4 + 4, :],
                            pkt_out[it][h]
                            [s * PKT + V_OFF:s * PKT + V_OFF + V_SZ]
                            .rearrange("(c p h) -> p c h", p=128, c=4))

                # ---- issue dmpnn gathers early (gpsimd) so they overlap the
                # ---- attention below (tensor engine); first-half bonds first
                Tn = pc.tile([128, BS // 128, H], bf16, name="Tn")
                for h in range(2):
                    for cc in range(4):
                        c = h * 4 + cc
                        nc.gpsimd.indirect_dma_start(
                            out=Tn[:, c, :], out_offset=None, in_=mnall,
                            in_offset=bass.IndirectOffsetOnAxis(
                                ap=NbIdx[:, c, 0:1], axis=0))
                    nc.vector.tensor_scalar_mul(Tn[:, bass.ts(h, 4), :],
                                                Tn[:, bass.ts(h, 4), :], -1.0)
                    for j in range(1, 7):
                        for cc in range(4):
                            c = h * 4 + cc
                            nc.gpsimd.indirect_dma_start(
                                out=Tn[:, c, :], out_offset=None, in_=mnall,
                                in_offset=bass.IndirectOffsetOnAxis(
                                    ap=NbIdx[:, c, j:j + 1], axis=0),
                                compute_op=ALU.add)

                # ---- attention (rows = shard), flash-style over j blocks,
                # ---- interleaved with the per-half tail so each half's
                # ---- exchange collective hides under the other half's rows
                patt = tc.alloc_tile_pool(name=f"patt{it}", bufs=1)
                AttA = patt.tile([128, BS // 128, H], f32r, name="AttA")
                AtT = patt.tile([128, 2, BS], f32r, name="AtT")
                AmT = patt.tile([128, 2, BS], f32r, name="AmT")
                TT = pc.tile([128, 2, BS], f32r, name="TT")
                DmT = pc.tile([128, 2, BS], f32r, name="DmT")
                Alp = pc.tile([1, BS], f32r, name="Alp")
                AlB = pc.tile([128, BS], f32, name="AlB")
                D1 = pc.tile([128, BS], f32, name="D1")

                def flash_rows(ic):
                    pvp = ppsP.tile([128, 4, 512], f32, name="psP")
                    prev = None
                    for jx in range(B // 128 + 1):
                        if jx < B // 128:
                            j = J_ORDER[jx]
                            sp = ppsS.tile([128, 512], f32, name="psS")
                            for kt in range(2):
                                nc.tensor.matmul(sp[:],
                                                 KT[:, kt, bass.ts(j, 128)],
                                                 QT[:, kt, bass.ts(ic, 512)],
                                                 start=(kt == 0),
                                                 stop=(kt == 1))
                            pt = pmt.tile([128, 512], bf16, name="PT")
                            nc.scalar.activation(pt[:], sp[:], AF.Exp,
                                                 scale=SCALE_BOND)
                        if prev is not None:
                            pj, ppt = prev
                            for isub in range(4):
                                nc.tensor.matmul(
                                    pvp[:, isub, 0:H + 1],
                                    ppt[:, bass.ts(isub, 128)],
                                    Vb[:, pj, 0:H + 1],
                                    start=(pj == J_ORDER[0]),
                                    stop=(pj == J_ORDER[-1]))
                        if jx < B // 128:
                            prev = (j, pt)
                    for isub in range(4):
                        rec = pmt.tile([128, 1], f32, name="rec")
                        nc.vector.reciprocal(rec[:], pvp[:, isub, H:H + 1])
                        nc.vector.tensor_scalar_mul(
                            AttA[:, ic * 4 + isub, :], pvp[:, isub, 0:H],
                            rec[:])

                def tail_half(h):
                    """att_msgT + dmpnn + alpha + combine for one bond half.

                    The dmpnn transposes read IdentF, an identity copy with an
                    artificial zero-valued dependency on this half's last
                    attention output, so the compile-time scheduler cannot
                    hoist this gather-dependent block into the attention
                    stream (that would stall the tensor engine on the slow
                    gathers)."""
                    hc = bass.ts(h, HB)
                    Zf = pmt.tile([128, 1], f32, name="Zf")
                    nc.vector.tensor_scalar_mul(
                        Zf[:], AttA[:, h * 4 + 3, 0:1].bitcast(f32), 0.0)
                    IdentF = pc.tile([128, 128], bf16, name=f"IdentF{h}")
                    nc.vector.tensor_scalar_add(IdentF[:], IdentB[:], Zf[:])
                    for ht in range(2):
                        pst = ppsA.tile([128, 512], f32, name="psA")
                        for cc in range(4):
                            c = h * 4 + cc
                            nc.tensor.transpose(
                                pst[:, bass.ts(cc, 128)],
                                AttA[:, c, bass.ts(ht, 128)].bitcast(f32),
                                Ident[:])
                        nc.vector.tensor_copy(AtT[:, ht, hc], pst[:])
                    for ht in range(2):
                        ps = ppsA.tile([128, 512], f32, name="psA")
                        for kt in range(2):
                            nc.tensor.matmul(ps[:], Wa[:, kt, bass.ts(ht, 128)],
                                             AtT[:, kt, hc],
                                             start=(kt == 0), stop=(kt == 1))
                        nc.vector.tensor_copy(AmT[:, ht, hc], ps[:])
                    for ht in range(2):
                        pst = ppsA.tile([128, 512], bf16, name="psA")
                        for cc in range(4):
                            c = h * 4 + cc
                            nc.tensor.transpose(
                                pst[:, bass.ts(cc, 128)],
                                Tn[:, c, bass.ts(ht, 128)],
                                IdentF[:])
                        nc.vector.tensor_copy(TT[:, ht, hc], pst[:])
                    for ht in range(2):
                        ps = ppsA.tile([128, 512], f32, name="psA")
                        for kt in range(2):
                            nc.tensor.matmul(ps[:], Wh[:, kt, bass.ts(ht, 128)],
                                             TT[:, kt, hc],
                                             start=(kt == 0), stop=(kt == 1))
                        nc.vector.tensor_copy(DmT[:, ht, hc], ps[:])
                    ps = ppsA.tile([1, 512], f32, name="psA")
                    nc.tensor.matmul(ps[:], Wal1[:, 0, :], DmT[:, 0, hc],
                                     start=True, stop=False)
                    nc.tensor.matmul(ps[:], Wal1[:, 1, :], DmT[:, 1, hc],
                                     start=False, stop=False)
                    nc.tensor.matmul(ps[:], Wal2[:, 0, :], AmT[:, 0, hc],
                                     start=False, stop=False)
                    nc.tensor.matmul(ps[:], Wal2[:, 1, :], AmT[:, 1, hc],
                                     start=False, stop=True)
                    nc.scalar.activation(Alp[:, hc], ps[:], AF.Sigmoid,
                                         bias=Walb[:])
                    ps = ppsA.tile([128, 512], f32, name="psA")
                    nc.tensor.matmul(ps[:], Onesr[:], Alp[:, hc],
                                     start=True, stop=True)
                    nc.vector.tensor_copy(AlB[:, hc], ps[:])
                    for ht in range(2):
                        d1 = D1[:, hc]
                        nc.vector.tensor_tensor(d1, DmT[:, ht, hc],
                                                AmT[:, ht, hc],
                                                op=ALU.subtract)
                        nc.vector.tensor_tensor(d1, d1, AlB[:, hc],
                                                op=ALU.mult)
                        nc.vector.tensor_tensor(d1, d1, AmT[:, ht, hc],
                                                op=ALU.add)
                        nc.vector.tensor_tensor(d1, d1, InpT[:, ht, hc],
                                                op=ALU.add)
                        nc.scalar.activation(MsgTx[:, ht, hc], d1, AF.Relu)

                for h in range(2):
                    flash_rows(h)
                    tail_half(h)
                    if it + 1 < NITER:
                        emit_exchange_half(it + 1, h, ppsA, "psA")
                patt.release()
                emit_agn(agn_ins[it + 1], agn_outs[it + 1], ppsA, "psA", pmt)

        # ============ Readout + per-molecule pooling =======================
        with tc.tile_pool(name="pd", bufs=1) as pd, \
             tc.tile_pool(name="pd2", bufs=2) as pd2, \
             tc.tile_pool(name="pd_ps", bufs=4, space="PSUM") as pdps:
            AmN = pd.tile([128, AS // 128, H], bf16, name="AmN")
            for c in range(AS // 128):
                nc.gpsimd.indirect_dma_start(
                    out=AmN[:, c, :], out_offset=None, in_=agn_outs[NITER][:],
                    in_offset=bass.IndirectOffsetOnAxis(
                        ap=RoIdx[:, c, 0:1], axis=0))
            for j in range(1, MNB):
                for c in range(AS // 128):
                    nc.gpsimd.indirect_dma_start(
                        out=AmN[:, c, :], out_offset=None, in_=agn_outs[NITER][:],
                        in_offset=bass.IndirectOffsetOnAxis(
                            ap=RoIdx[:, c, j:j + 1], axis=0),
                        compute_op=ALU.add)
            AmT2 = pd.tile([128, 2, AS], f32r, name="AmT2")
            for ht in range(2):
                pst = pdps.tile([128, AS], bf16, name="ps")
                for c in range(AS // 128):
                    nc.tensor.transpose(pst[:, bass.ts(c, 128)],
                                        AmN[:, c, bass.ts(ht, 128)],
                                        IdentB[:])
                nc.vector.tensor_copy(AmT2[:, ht, :], pst[:])
            Hm = pd.tile([128, AS // 128, H], f32r, name="Hm")
            for c in range(AS // 128):
                ps = pdps.tile([128, H], f32, name="ps")
                nc.tensor.matmul(ps[:], FeT_h[:, bass.ts(c, 128)], Wof_h[:],
                                 start=True, stop=False)
                nc.tensor.matmul(ps[:], FeT_l[:, bass.ts(c, 128)], Wof_l[:],
                                 start=False, stop=False)
                for kt in range(2):
                    nc.tensor.matmul(ps[:], AmT2[:, kt, bass.ts(c, 128)],
                                     Wom[:, kt, :], start=False, stop=False)
                nc.tensor.matmul(ps[:], Onesr[:, 0:128], Wob[:],
                                 start=False, stop=True)
                nc.scalar.activation(Hm[:, c, :], ps[:], AF.Relu)
            HmT = pd.tile([128, 2, AS], f32r, name="HmT")
            for ht in range(2):
                pst = pdps.tile([128, AS], f32, name="ps")
                for c in range(AS // 128):
                    nc.tensor.transpose(pst[:, bass.ts(c, 128)],
                                        Hm[:, c, bass.ts(ht, 128)].bitcast(f32),
                                        Ident[:])
                nc.vector.tensor_copy(HmT[:, ht, :], pst[:])
            # hm in per-molecule base-0 layout via SBUF->SBUF DMA
            HmM = pd.tile([64, MPC, H], f32r, name="HmM")
            nc.sync.dma_start(HmM[:, 0:MPC:2, :], Hm[0:64, :, :])
            nc.sync.dma_start(HmM[:, 1:MPC:2, :], Hm[64:128, :, :])
            T2T = pd.tile([128, 2, AS], f32r, name="T2T")
            for ht in range(2):
                ps = pdps.tile([128, AS], f32, name="ps")
                for kt in range(2):
                    nc.tensor.matmul(ps[:], Wa[:, kt, bass.ts(ht, 128)],
                                     HmT[:, kt, :], start=(kt == 0), stop=(kt == 1))
                nc.vector.tensor_copy(T2T[:, ht, :], ps[:])
            SC2 = pd.tile([64, MPC, 64], f32, name="SC2")
            for m in range(MPC):
                ps = pdps.tile([64, 64], f32, name="ps")
                for kt in range(2):
                    nc.tensor.matmul(ps[:], T2T[:, kt, bass.ts(m, 64)],
                                     HmT[:, kt, bass.ts(m, 64)],
                                     start=(kt == 0), stop=(kt == 1))
                nc.vector.tensor_copy(SC2[:, m, :], ps[:])
            Mx2 = pd.tile([64, MPC, 1], f32, name="Mx2")
            nc.vector.tensor_reduce(Mx2[:], SC2[:], axis=AX.X, op=ALU.max)
            NMx2 = pd.tile([64, MPC, 1], f32, name="NMx2")
            nc.vector.tensor_scalar_mul(NMx2[:], Mx2[:], -1.0)
            E2 = pd.tile([64, MPC, 64], f32, name="E2")
            for m in range(MPC):
                nc.scalar.activation(E2[:, m, :], SC2[:, m, :], AF.Exp,
                                     bias=NMx2[:, m, :])
            Sum2 = pd.tile([64, MPC, 1], f32, name="Sum2")
            RS2 = pd.tile([64, MPC, 1], f32, name="RS2")
            nc.vector.tensor_reduce(Sum2[:], E2[:], axis=AX.X, op=ALU.add)
            nc.vector.reciprocal(RS2[:], Sum2[:])
            BB2 = pd.tile([64, H], f32, name="BB2")
            psbb = pdps.tile([64, H], f32, name="ps")
            nc.tensor.matmul(psbb[:], Onesr[:, 0:64], Wbb[:], start=True, stop=True)
            nc.vector.tensor_copy(BB2[:], psbb[:])
            OutS = pd.tile([1, MPC, H], f32, name="OutS")
            for m in range(MPC):
                pst = pdps.tile([64, 64], f32, name="ps")
                nc.tensor.transpose(pst[:], E2[:, m, :], Ident[0:64, 0:64])
                E2T = pd2.tile([64, 64], f32r, name="E2T")
                nc.vector.tensor_copy(E2T[:], pst[:])
                UT = pd2.tile([128, 2, 64], f32r, name="UT")
                for hs in range(2):
                    psu = pdps.tile([128, 64], f32, name="ps")
                    nc.tensor.matmul(psu[:], HmM[:, m, bass.ts(hs, 128)], E2T[:],
                                     start=True, stop=True)
                    nc.vector.tensor_copy(UT[:, hs, :], psu[:])
                psb = pdps.tile([64, H], f32, name="ps")
                for kt in range(2):
                    nc.tensor.matmul(psb[:], UT[:, kt, :], Wbw[:, kt, :],
                                     start=(kt == 0), stop=(kt == 1))
                AH = pd2.tile([64, H], f32, name="AH")
                nc.vector.tensor_scalar_mul(AH[:], psb[:], RS2[:, m, :])
                nc.vector.tensor_tensor(AH[:], AH[:], BB2[:], op=ALU.add)
                nc.vector.tensor_scalar_max(AH[:], AH[:], 0.0)
                XS = pd2.tile([64, H], f32r, name="XS")
                nc.vector.tensor_tensor(XS[:], AH[:], HmM[:, m, :], op=ALU.add)
                psm = pdps.tile([1, H], f32, name="ps")
                nc.tensor.matmul(psm[:], OnesC[0:64, :], XS[:],
                                 start=True, stop=True)
                nc.vector.tensor_scalar_mul(OutS[:, m, :], psm[:], 1.0 / APM)
            nc.sync.dma_start(out_mol[:].rearrange("(o m) h -> o m h", o=1), OutS[:])

    nc.compile()
    return nc


def _host_prepare(inputs):
    f_atoms = np.asarray(inputs["f_atoms"], np.float32)
    f_bonds = np.asarray(inputs["f_bonds"], np.float32)
    a2b = np.asarray(inputs["a2b"]).astype(np.int64)
    b2a = np.asarray(inputs["b2a"]).astype(np.int64)
    b2revb = np.asarray(inputs["b2revb"]).astype(np.int64)

    fbp = np.zeros((B, H), np.float32)
    fbp[:, :FD] = f_bonds
    fbT_full = np.ascontiguousarray(fbp.T)
    faT_full = np.ascontiguousarray(f_atoms.T)

    W = {k: np.asarray(inputs[k], np.float32) for k in
         ("Wq_atom", "Wk_atom", "Wv_atom", "Wi", "Wh", "Wq", "Wk", "Wv", "Wa",
          "Walpha_w", "Wo_w", "Wb_w")}
    wi_p = np.zeros((H, H), np.float32)
    wi_p[:FD, :] = W["Wi"]

    base = dict(
        wq_a=W["Wq_atom"], wk_a=W["Wk_atom"], wv_a=W["Wv_atom"],
        ln_g=np.asarray(inputs["ln_g"], np.float32).reshape(1, F),
        ln_b=np.asarray(inputs["ln_b"], np.float32).reshape(1, F),
        wi=wi_p, wh=W["Wh"], wq=W["Wq"], wk=W["Wk"], wv=W["Wv"], wa=W["Wa"],
        wal1=np.ascontiguousarray(W["Walpha_w"][:H]),
        wal2=np.ascontiguousarray(W["Walpha_w"][H:]),
        walb=np.asarray(inputs["Walpha_b"], np.float32).reshape(1, 1),
        wo_f=np.ascontiguousarray(W["Wo_w"][:F]),
        wo_m=np.ascontiguousarray(W["Wo_w"][F:]),
        wo_b=np.asarray(inputs["Wo_b"], np.float32).reshape(1, H),
        wb_w=W["Wb_w"],
        wb_b=np.asarray(inputs["Wb_b"], np.float32).reshape(1, H),
        ident=np.eye(128, dtype=np.float32),
        onesr=np.ones((1, 128), np.float32),
        onesc=np.ones((128, 1), np.float32),
    )

    in_maps = []
    for c in range(NC):
        bonds = np.arange(c * BS, (c + 1) * BS)
        # [7, BS]: row 0 = b2revb (subtracted), rows 1..6 = a2b[b2a]
        terms = np.stack([b2revb[bonds]]
                         + [a2b[b2a[bonds], j] for j in range(MNB)])
        nb2 = np.ascontiguousarray(
            terms.reshape(7, BS // 128, 128).transpose(2, 1, 0)).astype(np.int32)
        atoms = np.arange(c * AS, (c + 1) * AS)
        ro = np.stack([a2b[atoms, j] for j in range(MNB)])  # [6, AS]
        ro2 = np.ascontiguousarray(
            ro.reshape(MNB, AS // 128, 128).transpose(2, 1, 0)).astype(np.int32)
        m = dict(base)
        m["fbT"] = np.ascontiguousarray(fbT_full[:, bonds])
        m["faT"] = np.ascontiguousarray(faT_full[:, atoms])
        m["fa"] = np.ascontiguousarray(f_atoms[atoms])
        m["nb_idx"] = nb2
        m["ro_idx"] = ro2
        in_maps.append(m)
    return in_maps


def kernel(**inputs):
    if "nc" not in _CACHE:
        _CACHE["nc"] = _build()
    nc = _CACHE["nc"]
    in_maps = _host_prepare(inputs)
    res = run_bass_kernel_spmd(nc, in_maps, core_ids=list(range(NC)))
    out = np.concatenate([res.results[c]["mol_out"] for c in range(NC)], 0)
    return np.ascontiguousarray(out.astype(np.float32))


# revision 62
# speedup vs baseline: 1.1566x; 1.1000x over previous
"""Trainium2 Bass kernel for nn_HGNNEncoder (DMPNN + global bond attention).

Sharding: data-parallel over bonds/atoms/molecules across 8 NeuronCores.
The global bond-bond attention stays exact: each core computes its 1024-row
slice of the 8192x8192 attention (bf16 QK^T, exp on ScalarE with the 1/16
scale folded in, bf16 P@[V|1] with fused row-sum column, normalized
post-hoc).  Between iterations each core computes its K^T / V column shard
locally and the shards are exchanged with half-sized bf16 AllGathers (so
attention on first-half columns starts while the second half is in flight);
the natural-layout message is AllGathered in bf16 for the DMPNN row-gathers,
which run on GpSimd fully overlapped with the attention.
"""

import numpy as np

import concourse.bass as bass
import concourse.bacc as bacc
import concourse.mybir as mybir
import concourse.tile as tile
from concourse.bass_utils import run_bass_kernel_spmd

NC = 8
B, NA, MNB = 8192, 4096, 6
H = 256
F = 133
FD = 147
BS = B // NC          # 1024 bonds per core
AS = NA // NC         # 512 atoms per core
APM = 64              # atoms per molecule
MPC = AS // APM       # 8 molecules per core
NITER = 3

f32 = mybir.dt.float32
f32r = mybir.dt.float32r
bf16 = mybir.dt.bfloat16
AF = mybir.ActivationFunctionType
ALU = mybir.AluOpType
AX = mybir.AxisListType

SCALE_BOND = float(1.0 / np.sqrt(np.float32(H)))
SCALE_ATOM = float(1.0 / np.sqrt(np.float32(F)))

# flash-loop column-block order: every core exchanges the first half of its
# bond shard first, so all first-half column blocks (j%8 < 4) arrive first
J_ORDER = [j for j in range(B // 128) if j % 8 < 4] + \
          [j for j in range(B // 128) if j % 8 >= 4]

_CACHE = {}


def _build():
    nc = bacc.Bacc("TRN2", target_bir_lowering=False, debug=False, num_devices=NC)

    def inp(name, shape, dt=f32):
        return nc.dram_tensor(name, list(shape), dt, kind="ExternalInput")

    fbT = inp("fbT", [H, BS])
    faT = inp("faT", [F, AS])
    fa = inp("fa", [AS, F])
    wq_a = inp("wq_a", [F, F]); wk_a = inp("wk_a", [F, F]); wv_a = inp("wv_a", [F, F])
    ln_g = inp("ln_g", [1, F]); ln_b = inp("ln_b", [1, F])
    wi = inp("wi", [H, H])
    wh = inp("wh", [H, H]); wq = inp("wq", [H, H]); wk = inp("wk", [H, H])
    wv = inp("wv", [H, H]); wa = inp("wa", [H, H])
    wal1 = inp("wal1", [H, 1]); wal2 = inp("wal2", [H, 1]); walb = inp("walb", [1, 1])
    wo_f = inp("wo_f", [F, H]); wo_m = inp("wo_m", [H, H]); wo_b = inp("wo_b", [1, H])
    wb_w = inp("wb_w", [H, H]); wb_b = inp("wb_b", [1, H])
    ident_in = inp("ident", [128, 128])
    onesr_in = inp("onesr", [1, 128])
    onesc_in = inp("onesc", [128, 1])
    nb_idx = inp("nb_idx", [128, BS // 128, 7], mybir.dt.int32)
    ro_idx = inp("ro_idx", [128, AS // 128, MNB], mybir.dt.int32)
    out_mol = nc.dram_tensor("mol_out", [MPC, H], f32, kind="ExternalOutput")

    with tile.TileContext(nc) as tc, \
         tc.tile_pool(name="persist", bufs=1) as per, \
         tc.tile_pool(name="dram", bufs=1, space="DRAM") as dram:

        def load_round(pool, src_ap, shape, name, dt=f32r, raw_pool=None):
            raw = (raw_pool or pool).tile(list(shape), f32, name=name + "_raw")
            nc.sync.dma_start(raw[:], src_ap)
            if dt == f32:
                return raw
            t = pool.tile(list(shape), dt, name=name)
            nc.vector.tensor_copy(t[:], raw[:])
            return t

        # ---------------- persistent weights ----------------
        with tc.tile_pool(name="raws", bufs=1) as raws:
            def loadw(src, name, cols=H):
                return load_round(per, src[:].rearrange("(t p) h -> p t h", p=128),
                                  [128, 2, cols], name, raw_pool=raws)
            Wi = loadw(wi, "Wi"); Wh = loadw(wh, "Wh"); Wq = loadw(wq, "Wq")
            Wk = loadw(wk, "Wk"); Wv = loadw(wv, "Wv"); Wa = loadw(wa, "Wa")
            Wom = loadw(wo_m, "Wom"); Wbw = loadw(wb_w, "Wbw")
            Wal1 = loadw(wal1, "Wal1", cols=1); Wal2 = loadw(wal2, "Wal2", cols=1)
            Wof_h = load_round(per, wo_f[0:128, :], [128, H], "Wof_h", raw_pool=raws)
            Wof_l = load_round(per, wo_f[128:F, :], [5, H], "Wof_l", raw_pool=raws)
            Wob = load_round(per, wo_b[:], [1, H], "Wob", raw_pool=raws)
            Wbb = load_round(per, wb_b[:], [1, H], "Wbb", raw_pool=raws)
            Ident = load_round(per, ident_in[:], [128, 128], "Ident", dt=f32)
            IdentB = load_round(per, ident_in[:], [128, 128], "IdentB", dt=bf16,
                                raw_pool=raws)
            Onesr = load_round(per, onesr_in[:], [1, 128], "Onesr", raw_pool=raws)
            OnesC = load_round(per, onesc_in[:], [128, 1], "OnesC", raw_pool=raws)
            Walb = load_round(per, walb[:], [1, 1], "Walb", dt=f32)
            OnesrF = load_round(per, onesr_in[:], [1, 128], "OnesrF", dt=f32)
            NbIdx = per.tile([128, BS // 128, 7], mybir.dt.int32, name="NbIdx")
            nc.sync.dma_start(NbIdx[:], nb_idx[:])
            RoIdx = per.tile([128, AS // 128, MNB], mybir.dt.int32, name="RoIdx")
            nc.sync.dma_start(RoIdx[:], ro_idx[:])


        # persistent activations
        InpT = per.tile([128, 2, BS], f32r, name="InpT")
        KT = per.tile([128, 2, B], bf16, name="KT")
        Vb = per.tile([128, B // 128, H + 1], bf16, name="Vb")
        QT = per.tile([128, 2, BS], bf16, name="QT")
        MsgTx = per.tile([128, 2, BS], f32r, name="MsgTx")   # local shard msgT
        KTl = per.tile([128, 2, BS], bf16, name="KTl")       # local K^T shard
        Vl = per.tile([128, BS // 128, H + 1], bf16, name="Vl")  # local V shard
        FeT_h = per.tile([128, AS], f32r, name="FeT_h")
        FeT_l = per.tile([5, AS], f32r, name="FeT_l")
        nc.vector.memset(Vl[:], 1.0)   # ones column persists across iters

        # packed per-core half-shard attention payload: [K^T 256x512 | V 512x257]
        HB = BS // 2
        KT_OFF = 0
        KT_SZ = H * HB                    # 131072
        V_OFF = KT_SZ
        V_SZ = HB * (H + 1)               # 131584
        PKT = V_OFF + V_SZ                # 262656
        pkt_in = [[dram.tile([PKT], bf16, name=f"pkt_in{t}_{h}")
                   for h in range(2)] for t in range(NITER)]
        pkt_out = [[dram.tile([NC * PKT], bf16, name=f"pkt_out{t}_{h}",
                              addr_space="Shared") for h in range(2)]
                   for t in range(NITER)]
        agn_ins = [dram.tile([BS, H], bf16, name=f"agn_in{t}")
                   for t in range(NITER + 1)]
        agn_outs = [dram.tile([B, H], bf16, name=f"agn_out{t}",
                              addr_space="Shared") for t in range(NITER + 1)]
        RG = [list(range(NC))]

        def emit_exchange_half(t, half, psum_pool, tag):
            """K^T/V for one bond-half of the shard, packed, one AllGather."""
            for ht in range(2):
                ps = psum_pool.tile([128, HB], f32, name=tag)
                for kt in range(2):
                    nc.tensor.matmul(ps[:], Wk[:, kt, bass.ts(ht, 128)],
                                     MsgTx[:, kt, bass.ts(half, HB)],
                                     start=(kt == 0), stop=(kt == 1))
                nc.vector.tensor_copy(KTl[:, ht, bass.ts(half, HB)], ps[:])
            nc.sync.dma_start(
                pkt_in[t][half][KT_OFF:KT_OFF + KT_SZ]
                .rearrange("(p t i) -> p t i", p=128, t=2),
                KTl[:, :, bass.ts(half, HB)])
            for cc in range(4):
                c = half * 4 + cc
                ps = psum_pool.tile([128, H], f32, name=tag)
                for kt in range(2):
                    nc.tensor.matmul(ps[:], MsgTx[:, kt, bass.ts(c, 128)],
                                     Wv[:, kt, :],
                                     start=(kt == 0), stop=(kt == 1))
                nc.vector.tensor_copy(Vl[:, c, 0:H], ps[:])
            nc.sync.dma_start(
                pkt_in[t][half][V_OFF:V_OFF + V_SZ]
                .rearrange("(c p h) -> p c h", p=128, c=4),
                Vl[:, bass.ts(half, 4), :])
            nc.gpsimd.collective_compute(
                "AllGather", ALU.bypass, RG,
                ins=[pkt_in[t][half].opt()], outs=[pkt_out[t][half].opt()])

        def emit_agn(dst_in, dst_out, psum_pool, tag, natc_pool):
            # natural layout via PE transposes, DMA per 128-bond block
            for c in range(BS // 128):
                psn = psum_pool.tile([128, H], f32, name=tag)
                for ht in range(2):
                    nc.tensor.transpose(
                        psn[:, bass.ts(ht, 128)],
                        MsgTx[:, ht, bass.ts(c, 128)].bitcast(f32), Ident[:])
                natc = natc_pool.tile([128, H], bf16, name="natc")
                nc.vector.tensor_copy(natc[:], psn[:])
                nc.sync.dma_start(dst_in[bass.ts(c, 128), :], natc[:])
            nc.gpsimd.collective_compute(
                "AllGather", ALU.bypass, RG,
                ins=[dst_in.opt()], outs=[dst_out.opt()])

        # ============ Phase B: message_0 = relu(f_bonds @ Wi) ==============
        with tc.tile_pool(name="pb", bufs=1) as pb, \
             tc.tile_pool(name="pb2", bufs=2) as pb2, \
             tc.tile_pool(name="pb_ps", bufs=4, space="PSUM") as pbps:
            FbT = load_round(pb, fbT[:].rearrange("(t p) i -> p t i", p=128),
                             [128, 2, BS], "FbT")
            for ht in range(2):
                for ch in range(BS // 512):
                    ps = pbps.tile([128, 512], f32, name="ps")
                    for kt in range(2):
                        nc.tensor.matmul(ps[:], Wi[:, kt, bass.ts(ht, 128)],
                                         FbT[:, kt, bass.ts(ch, 512)],
                                         start=(kt == 0), stop=(kt == 1))
                    nc.vector.tensor_copy(InpT[:, ht, bass.ts(ch, 512)], ps[:])
                    nc.scalar.activation(MsgTx[:, ht, bass.ts(ch, 512)], ps[:],
                                         AF.Relu)
            # agn first: iteration 0's gathers depend on it and start ~60us
            # earlier this way, hiding under iteration 0's attention
            emit_agn(agn_ins[0], agn_outs[0], pbps, "ps", pb2)
            emit_exchange_half(0, 0, pbps, "ps")
            emit_exchange_half(0, 1, pbps, "ps")

        # ============ Phase A: per-molecule atom self-attention ============
        with tc.tile_pool(name="pa", bufs=1) as pa, \
             tc.tile_pool(name="pa2", bufs=2) as pa2, \
             tc.tile_pool(name="pa_ps", bufs=4, space="PSUM") as paps:
            WqA_h = load_round(pa, wq_a[0:128, :], [128, F], "WqA_h", dt=f32)
            WqA_l = load_round(pa, wq_a[128:F, :], [5, F], "WqA_l", dt=f32)
            WkA_h = load_round(pa, wk_a[0:128, :], [128, F], "WkA_h", dt=f32)
            WkA_l = load_round(pa, wk_a[128:F, :], [5, F], "WkA_l", dt=f32)
            WvA_h = load_round(pa, wv_a[0:128, :], [128, F], "WvA_h", dt=f32)
            WvA_l = load_round(pa, wv_a[128:F, :], [5, F], "WvA_l", dt=f32)
            LnG = load_round(pa, ln_g[:], [1, F], "LnG", dt=f32)
            LnB = load_round(pa, ln_b[:], [1, F], "LnB", dt=f32)
            FaT_h = load_round(pa, faT[0:128, :], [128, AS], "FaT_h", dt=f32)
            FaT_l = load_round(pa, faT[128:F, :], [5, AS], "FaT_l", dt=f32)
            XN = pa.tile([64, MPC, F], f32, name="XN")
            nc.sync.dma_start(XN[:], fa[:].rearrange("(m a) f -> a m f", a=64))
            GB = pa.tile([64, F], f32, name="GB")
            BB = pa.tile([64, F], f32, name="BB")
            for bc_src, bc_dst in ((LnG, GB), (LnB, BB)):
                ps = paps.tile([64, F], f32, name="ps")
                nc.tensor.matmul(ps[:], OnesrF[:, 0:64], bc_src[:],
                                 start=True, stop=True)
                nc.vector.tensor_copy(bc_dst[:], ps[:])

            def mm133(dst, lhs_pair, rhs_pair, n):
                ps = paps.tile([dst.shape[0], n], f32, name="ps")
                nc.tensor.matmul(ps[:], lhs_pair[0], rhs_pair[0],
                                 start=True, stop=False)
                nc.tensor.matmul(ps[:], lhs_pair[1], rhs_pair[1],
                                 start=False, stop=True)
                nc.vector.tensor_copy(dst, ps[:])

            QTa_h = pa.tile([128, AS], f32, name="QTa_h")
            QTa_l = pa.tile([5, AS], f32, name="QTa_l")
            KTa_h = pa.tile([128, AS], f32, name="KTa_h")
            KTa_l = pa.tile([5, AS], f32, name="KTa_l")
            mm133(QTa_h[:], (WqA_h[:, 0:128], WqA_l[:, 0:128]),
                  (FaT_h[:], FaT_l[:]), AS)
            mm133(QTa_l[:], (WqA_h[:, 128:F], WqA_l[:, 128:F]),
                  (FaT_h[:], FaT_l[:]), AS)
            mm133(KTa_h[:], (WkA_h[:, 0:128], WkA_l[:, 0:128]),
                  (FaT_h[:], FaT_l[:]), AS)
            mm133(KTa_l[:], (WkA_h[:, 128:F], WkA_l[:, 128:F]),
                  (FaT_h[:], FaT_l[:]), AS)
            VN = pa.tile([64, MPC, F], f32, name="VN")
            for m in range(MPC):
                mm133(VN[:, m, :],
                      (FaT_h[:, bass.ts(m, 64)], FaT_l[:, bass.ts(m, 64)]),
                      (WvA_h[:], WvA_l[:]), F)
            E = pa.tile([64, MPC, 64], f32, name="E")
            for m in range(MPC):
                ps = paps.tile([64, 64], f32, name="ps")
                nc.tensor.matmul(ps[:], QTa_h[:, bass.ts(m, 64)],
                                 KTa_h[:, bass.ts(m, 64)], start=True, stop=False)
                nc.tensor.matmul(ps[:], QTa_l[:, bass.ts(m, 64)],
                                 KTa_l[:, bass.ts(m, 64)], start=False, stop=True)
                nc.scalar.activation(E[:, m, :], ps[:], AF.Exp, scale=SCALE_ATOM)
            SumsA = pa.tile([64, MPC, 1], f32, name="SumsA")
            RS = pa.tile([64, MPC, 1], f32, name="RS")
            nc.vector.tensor_reduce(SumsA[:], E[:], axis=AX.X, op=ALU.add)
            nc.vector.reciprocal(RS[:], SumsA[:])
            AttnN = pa.tile([64, MPC, F], f32, name="AttnN")
            for m in range(MPC):
                pst = paps.tile([64, 64], f32, name="ps")
                nc.tensor.transpose(pst[:], E[:, m, :], Ident[0:64, 0:64])
                ET = pa2.tile([64, 64], f32, name="ET")
                nc.vector.tensor_copy(ET[:], pst[:])
                ps = paps.tile([64, F], f32, name="ps")
                nc.tensor.matmul(ps[:], ET[:], VN[:, m, :], start=True, stop=True)
                nc.vector.tensor_scalar_mul(AttnN[:, m, :], ps[:], RS[:, m, :])
            SumX = pa.tile([64, MPC, F], f32, name="SumX")
            nc.vector.tensor_tensor(SumX[:], XN[:], AttnN[:], op=ALU.add)
            Mu = pa.tile([64, MPC, 1], f32, name="Mu")
            nc.vector.tensor_reduce(Mu[:], SumX[:], axis=AX.X, op=ALU.add)
            nc.vector.tensor_scalar_mul(Mu[:], Mu[:], 1.0 / F)
            XC = pa.tile([64, MPC, F], f32, name="XC")
            for m in range(MPC):
                nc.vector.tensor_scalar_sub(XC[:, m, :], SumX[:, m, :], Mu[:, m, :])
            SQ = pa.tile([64, MPC, F], f32, name="SQ")
            nc.vector.tensor_tensor(SQ[:], XC[:], XC[:], op=ALU.mult)
            Var = pa.tile([64, MPC, 1], f32, name="Var")
            nc.vector.tensor_reduce(Var[:], SQ[:], axis=AX.X, op=ALU.add)
            Std = pa.tile([64, MPC, 1], f32, name="Std")
            EpsT = pa.tile([64, 1], f32, name="EpsT")
            nc.vector.memset(EpsT[:], 1e-5)
            nc.scalar.activation(Std[:], Var[:], AF.Sqrt, scale=1.0 / F,
                                 bias=EpsT[:])
            RStd = pa.tile([64, MPC, 1], f32, name="RStd")
            nc.vector.reciprocal(RStd[:], Std[:])
            FeN = pa.tile([64, MPC, F], f32, name="FeN")
            for m in range(MPC):
                nc.vector.tensor_scalar_mul(XC[:, m, :], XC[:, m, :], RStd[:, m, :])
                nc.vector.tensor_tensor(XC[:, m, :], XC[:, m, :], GB[:], op=ALU.mult)
                nc.vector.tensor_tensor(FeN[:, m, :], XC[:, m, :], BB[:], op=ALU.add)
            for m in range(MPC):
                ps1 = paps.tile([128, 64], f32, name="ps")
                nc.tensor.transpose(ps1[:], FeN[:, m, 0:128], Ident[0:64, 0:64])
                nc.vector.tensor_copy(FeT_h[:, bass.ts(m, 64)], ps1[:])
                ps2 = paps.tile([5, 64], f32, name="ps")
                nc.tensor.transpose(ps2[:], FeN[:, m, 128:F], Ident[0:64, 0:64])
                nc.vector.tensor_copy(FeT_l[:, bass.ts(m, 64)], ps2[:])

        # ============ Phase C: 3 message-passing iterations ================
        for it in range(NITER):
            mnall = agn_outs[it][:]
            with tc.tile_pool(name=f"pc{it}", bufs=1) as pc, \
                 tc.tile_pool(name=f"pm{it}", bufs=2) as pmt, \
                 tc.tile_pool(name=f"pcA{it}", bufs=2, space="PSUM") as ppsA, \
                 tc.tile_pool(name=f"pcS{it}", bufs=2, space="PSUM") as ppsS, \
                 tc.tile_pool(name=f"pcP{it}", bufs=1, space="PSUM") as ppsP:

                # ---- QT from the local shard (no collective dependency)
                for ch in range(2):
                    for ht in range(2):
                        ps = ppsA.tile([128, 512], f32, name="psA")
                        for kt in range(2):
                            nc.tensor.matmul(ps[:], Wq[:, kt, bass.ts(ht, 128)],
                                             MsgTx[:, kt, bass.ts(ch, 512)],
                                             start=(kt == 0), stop=(kt == 1))
                        nc.vector.tensor_copy(QT[:, ht, bass.ts(ch, 512)], ps[:])

                # ---- load gathered K^T / V shards from the half payloads
                for h in range(2):
                    for s in range(NC):
                        nc.sync.dma_start(
                            KT[:, :, s * BS + h * HB:s * BS + (h + 1) * HB],
                            pkt_out[it][h]
                            [s * PKT + KT_OFF:s * PKT + KT_OFF + KT_SZ]
                            .rearrange("(p t i) -> p t i", p=128, t=2))
                        nc.sync.dma_start(
                            Vb[:, s * 8 + h * 4:s * 8 + h * 4 + 4, :],
                            pkt_out[it][h]
                            [s * PKT + V_OFF:s * PKT + V_OFF + V_SZ]
                            .rearrange("(c p h) -> p c h", p=128, c=4))

                # ---- issue dmpnn gathers early (gpsimd) so they overlap the
                # ---- attention below (tensor engine); first-half bonds first
                Tn = pc.tile([128, BS // 128, H], bf16, name="Tn")
                for h in range(2):
                    for cc in range(4):
                        c = h * 4 + cc
                        nc.gpsimd.indirect_dma_start(
                            out=Tn[:, c, :], out_offset=None, in_=mnall,
                            in_offset=bass.IndirectOffsetOnAxis(
                                ap=NbIdx[:, c, 0:1], axis=0))
                    nc.vector.tensor_scalar_mul(Tn[:, bass.ts(h, 4), :],
                                                Tn[:, bass.ts(h, 4), :], -1.0)
                    for j in range(1, 7):
                        for cc in range(4):
                            c = h * 4 + cc
                            nc.gpsimd.indirect_dma_start(
                                out=Tn[:, c, :], out_offset=None, in_=mnall,
                                in_offset=bass.IndirectOffsetOnAxis(
                                    ap=NbIdx[:, c, j:j + 1], axis=0),
                                compute_op=ALU.add)

                # ---- attention (rows = shard), flash-style over j blocks,
                # ---- interleaved with the per-half tail so each half's
                # ---- exchange collective hides under the other half's rows
                patt = tc.alloc_tile_pool(name=f"patt{it}", bufs=1)
                AttA = patt.tile([128, BS // 128, H], f32r, name="AttA")
                AtT = patt.tile([128, 2, BS], f32r, name="AtT")
                AmT = patt.tile([128, 2, BS], f32r, name="AmT")
                TT = pc.tile([128, 2, BS], f32r, name="TT")
                DmT = pc.tile([128, 2, BS], f32r, name="DmT")
                Alp = pc.tile([1, BS], f32r, name="Alp")
                AlB = pc.tile([128, BS], f32, name="AlB")
                D1 = pc.tile([128, BS], f32, name="D1")

                def flash_rows(ic):
                    pvp = ppsP.tile([128, 4, 512], f32, name="psP")
                    prev = None
                    for jx in range(B // 128 + 1):
                        if jx < B // 128:
                            j = J_ORDER[jx]
                            sp = ppsS.tile([128, 512], f32, name="psS")
                            for kt in range(2):
                                nc.tensor.matmul(sp[:],
                                                 KT[:, kt, bass.ts(j, 128)],
                                                 QT[:, kt, bass.ts(ic, 512)],
                                                 start=(kt == 0),
                                                 stop=(kt == 1))
                            pt = pmt.tile([128, 512], bf16, name="PT")
                            nc.scalar.activation(pt[:], sp[:], AF.Exp,
                                                 scale=SCALE_BOND)
                        if prev is not None:
                            pj, ppt = prev
                            for isub in range(4):
                                nc.tensor.matmul(
                                    pvp[:, isub, 0:H + 1],
                                    ppt[:, bass.ts(isub, 128)],
                                    Vb[:, pj, 0:H + 1],
                                    start=(pj == J_ORDER[0]),
                                    stop=(pj == J_ORDER[-1]))
                        if jx < B // 128:
                            prev = (j, pt)
                    for isub in range(4):
                        rec = pmt.tile([128, 1], f32, name="rec")
                        nc.vector.reciprocal(rec[:], pvp[:, isub, H:H + 1])
                        nc.vector.tensor_scalar_mul(
                            AttA[:, ic * 4 + isub, :], pvp[:, isub, 0:H],
                            rec[:])

                def tail_half(h):
                    """att_msgT + dmpnn + alpha + combine for one bond half.

                    The dmpnn transposes read IdentF, an identity copy with an
                    artificial zero-valued dependency on this half's last
                    attention output, so the compile-time scheduler cannot
                    hoist this gather-dependent block into the attention
                    stream (that would stall the tensor engine on the slow
                    gathers)."""
                    hc = bass.ts(h, HB)
                    Zf = pmt.tile([128, 1], f32, name="Zf")
                    nc.vector.tensor_scalar_mul(
                        Zf[:], AttA[:, h * 4 + 3, 0:1].bitcast(f32), 0.0)
                    IdentF = pc.tile([128, 128], bf16, name=f"IdentF{h}")
                    nc.vector.tensor_scalar_add(IdentF[:], IdentB[:], Zf[:])
                    for ht in range(2):
                        pst = ppsA.tile([128, 512], f32, name="psA")
                        for cc in range(4):
                            c = h * 4 + cc
                            nc.tensor.transpose(
                                pst[:, bass.ts(cc, 128)],
                                AttA[:, c, bass.ts(ht, 128)].bitcast(f32),
                                Ident[:])
                        nc.vector.tensor_copy(AtT[:, ht, hc], pst[:])
                    for ht in range(2):
                        ps = ppsA.tile([128, 512], f32, name="psA")
                        for kt in range(2):
                            nc.tensor.matmul(ps[:], Wa[:, kt, bass.ts(ht, 128)],
                                             AtT[:, kt, hc],
                                             start=(kt == 0), stop=(kt == 1))
                        nc.vector.tensor_copy(AmT[:, ht, hc], ps[:])
                    for ht in range(2):
                        pst = ppsA.tile([128, 512], bf16, name="psA")
                        for cc in range(4):
                            c = h * 4 + cc
                            nc.tensor.transpose(
                                pst[:, bass.ts(cc, 128)],
                                Tn[:, c, bass.ts(ht, 128)],
                                IdentF[:])
                        nc.vector.tensor_copy(TT[:, ht, hc], pst[:])
                    for ht in range(2):
                        ps = ppsA.tile([128, 512], f32, name="psA")
                        for kt in range(2):
                            nc.tensor.matmul(ps[:], Wh[:, kt, bass.ts(ht, 128)],
                                             TT[:, kt, hc],
                                             start=(kt == 0), stop=(kt == 1))
                        nc.vector.tensor_copy(DmT[:, ht, hc], ps[:])
                    ps = ppsA.tile([1, 512], f32, name="psA")
                    nc.tensor.matmul(ps[:], Wal1[:, 0, :], DmT[:, 0, hc],
                                     start=True, stop=False)
                    nc.tensor.matmul(ps[:], Wal1[:, 1, :], DmT[:, 1, hc],
                                     start=False, stop=False)
                    nc.tensor.matmul(ps[:], Wal2[:, 0, :], AmT[:, 0, hc],
                                     start=False, stop=False)
                    nc.tensor.matmul(ps[:], Wal2[:, 1, :], AmT[:, 1, hc],
                                     start=False, stop=True)
                    nc.scalar.activation(Alp[:, hc], ps[:], AF.Sigmoid,
                                         bias=Walb[:])
                    ps = ppsA.tile([128, 512], f32, name="psA")
                    nc.tensor.matmul(ps[:], Onesr[:], Alp[:, hc],
                                     start=True, stop=True)
                    nc.vector.tensor_copy(AlB[:, hc], ps[:])
                    for ht in range(2):
                        d1 = D1[:, hc]
                        nc.vector.tensor_tensor(d1, DmT[:, ht, hc],
                                                AmT[:, ht, hc],
                                                op=ALU.subtract)
                        nc.vector.tensor_tensor(d1, d1, AlB[:, hc],
                                                op=ALU.mult)
                        nc.vector.tensor_tensor(d1, d1, AmT[:, ht, hc],
                                                op=ALU.add)
                        nc.vector.tensor_tensor(d1, d1, InpT[:, ht, hc],
                                                op=ALU.add)
                        nc.scalar.activation(MsgTx[:, ht, hc], d1, AF.Relu)

                flash_rows(0)
                tail_half(0)
                if it + 1 < NITER:
                    emit_exchange_half(it + 1, 0, ppsA, "psA")
                flash_rows(1)
                tail_half(1)
                patt.release()
                # agn goes out BEFORE ktv-B: the next iteration's gathers
                # (~100us on gpsimd) depend on it and must finish before that
                # iteration's first tail; ktv-B is only needed ~55us in
                emit_agn(agn_ins[it + 1], agn_outs[it + 1], ppsA, "psA", pmt)
                if it + 1 < NITER:
                    emit_exchange_half(it + 1, 1, ppsA, "psA")

        # ============ Readout + per-molecule pooling =======================
        with tc.tile_pool(name="pd", bufs=1) as pd, \
             tc.tile_pool(name="pd2", bufs=2) as pd2, \
             tc.tile_pool(name="pd_ps", bufs=4, space="PSUM") as pdps:
            AmN = pd.tile([128, AS // 128, H], bf16, name="AmN")
            # c-outer so chunk c completes after 6 calls and its transpose +
            # Hm accumulation can start while later chunks still gather
            for c in range(AS // 128):
                nc.gpsimd.indirect_dma_start(
                    out=AmN[:, c, :], out_offset=None, in_=agn_outs[NITER][:],
                    in_offset=bass.IndirectOffsetOnAxis(
                        ap=RoIdx[:, c, 0:1], axis=0))
                for j in range(1, MNB):
                    nc.gpsimd.indirect_dma_start(
                        out=AmN[:, c, :], out_offset=None, in_=agn_outs[NITER][:],
                        in_offset=bass.IndirectOffsetOnAxis(
                            ap=RoIdx[:, c, j:j + 1], axis=0),
                        compute_op=ALU.add)
            AmT2 = pd.tile([128, 2, AS], f32r, name="AmT2")
            for ht in range(2):
                pst = pdps.tile([128, AS], bf16, name="ps")
                for c in range(AS // 128):
                    nc.tensor.transpose(pst[:, bass.ts(c, 128)],
                                        AmN[:, c, bass.ts(ht, 128)],
                                        IdentB[:])
                nc.vector.tensor_copy(AmT2[:, ht, :], pst[:])
            Hm = pd.tile([128, AS // 128, H], f32r, name="Hm")
            for c in range(AS // 128):
                ps = pdps.tile([128, H], f32, name="ps")
                nc.tensor.matmul(ps[:], FeT_h[:, bass.ts(c, 128)], Wof_h[:],
                                 start=True, stop=False)
                nc.tensor.matmul(ps[:], FeT_l[:, bass.ts(c, 128)], Wof_l[:],
                                 start=False, stop=False)
                for kt in range(2):
                    nc.tensor.matmul(ps[:], AmT2[:, kt, bass.ts(c, 128)],
                                     Wom[:, kt, :], start=False, stop=False)
                nc.tensor.matmul(ps[:], Onesr[:, 0:128], Wob[:],
                                 start=False, stop=True)
                nc.scalar.activation(Hm[:, c, :], ps[:], AF.Relu)
            HmT = pd.tile([128, 2, AS], f32r, name="HmT")
            for ht in range(2):
                pst = pdps.tile([128, AS], f32, name="ps")
                for c in range(AS // 128):
                    nc.tensor.transpose(pst[:, bass.ts(c, 128)],
                                        Hm[:, c, bass.ts(ht, 128)].bitcast(f32),
                                        Ident[:])
                nc.vector.tensor_copy(HmT[:, ht, :], pst[:])
            # hm in per-molecule base-0 layout via SBUF->SBUF DMA
            HmM = pd.tile([64, MPC, H], f32r, name="HmM")
            nc.sync.dma_start(HmM[:, 0:MPC:2, :], Hm[0:64, :, :])
            nc.sync.dma_start(HmM[:, 1:MPC:2, :], Hm[64:128, :, :])
            T2T = pd.tile([128, 2, AS], f32r, name="T2T")
            for ht in range(2):
                ps = pdps.tile([128, AS], f32, name="ps")
                for kt in range(2):
                    nc.tensor.matmul(ps[:], Wa[:, kt, bass.ts(ht, 128)],
                                     HmT[:, kt, :], start=(kt == 0), stop=(kt == 1))
                nc.vector.tensor_copy(T2T[:, ht, :], ps[:])
            SC2 = pd.tile([64, MPC, 64], f32, name="SC2")
            for m in range(MPC):
                ps = pdps.tile([64, 64], f32, name="ps")
                for kt in range(2):
                    nc.tensor.matmul(ps[:], T2T[:, kt, bass.ts(m, 64)],
                                     HmT[:, kt, bass.ts(m, 64)],
                                     start=(kt == 0), stop=(kt == 1))
                nc.vector.tensor_copy(SC2[:, m, :], ps[:])
            Mx2 = pd.tile([64, MPC, 1], f32, name="Mx2")
            nc.vector.tensor_reduce(Mx2[:], SC2[:], axis=AX.X, op=ALU.max)
            NMx2 = pd.tile([64, MPC, 1], f32, name="NMx2")
            nc.vector.tensor_scalar_mul(NMx2[:], Mx2[:], -1.0)
            E2 = pd.tile([64, MPC, 64], f32, name="E2")
            for m in range(MPC):
                nc.scalar.activation(E2[:, m, :], SC2[:, m, :], AF.Exp,
                                     bias=NMx2[:, m, :])
            Sum2 = pd.tile([64, MPC, 1], f32, name="Sum2")
            RS2 = pd.tile([64, MPC, 1], f32, name="RS2")
            nc.vector.tensor_reduce(Sum2[:], E2[:], axis=AX.X, op=ALU.add)
            nc.vector.reciprocal(RS2[:], Sum2[:])
            BB2 = pd.tile([64, H], f32, name="BB2")
            psbb = pdps.tile([64, H], f32, name="ps")
            nc.tensor.matmul(psbb[:], Onesr[:, 0:64], Wbb[:], start=True, stop=True)
            nc.vector.tensor_copy(BB2[:], psbb[:])
            OutS = pd.tile([1, MPC, H], f32, name="OutS")
            for m in range(MPC):
                pst = pdps.tile([64, 64], f32, name="ps")
                nc.tensor.transpose(pst[:], E2[:, m, :], Ident[0:64, 0:64])
                E2T = pd2.tile([64, 64], f32r, name="E2T")
                nc.vector.tensor_copy(E2T[:], pst[:])
                UT = pd2.tile([128, 2, 64], f32r, name="UT")
                for hs in range(2):
                    psu = pdps.tile([128, 64], f32, name="ps")
                    nc.tensor.matmul(psu[:], HmM[:, m, bass.ts(hs, 128)], E2T[:],
                                     start=True, stop=True)
                    nc.vector.tensor_copy(UT[:, hs, :], psu[:])
                psb = pdps.tile([64, H], f32, name="ps")
                for kt in range(2):
                    nc.tensor.matmul(psb[:], UT[:, kt, :], Wbw[:, kt, :],
                                     start=(kt == 0), stop=(kt == 1))
                AH = pd2.tile([64, H], f32, name="AH")
                nc.vector.tensor_scalar_mul(AH[:], psb[:], RS2[:, m, :])
                nc.vector.tensor_tensor(AH[:], AH[:], BB2[:], op=ALU.add)
                nc.vector.tensor_scalar_max(AH[:], AH[:], 0.0)
                XS = pd2.tile([64, H], f32r, name="XS")
                nc.vector.tensor_tensor(XS[:], AH[:], HmM[:, m, :], op=ALU.add)
                psm = pdps.tile([1, H], f32, name="ps")
                nc.tensor.matmul(psm[:], OnesC[0:64, :], XS[:],
                                 start=True, stop=True)
                nc.vector.tensor_scalar_mul(OutS[:, m, :], psm[:], 1.0 / APM)
            nc.sync.dma_start(out_mol[:].rearrange("(o m) h -> o m h", o=1), OutS[:])

    nc.compile()
    return nc


def _host_prepare(inputs):
    f_atoms = np.asarray(inputs["f_atoms"], np.float32)
    f_bonds = np.asarray(inputs["f_bonds"], np.float32)
    a2b = np.asarray(inputs["a2b"]).astype(np.int64)
    b2a = np.asarray(inputs["b2a"]).astype(np.int64)
    b2revb = np.asarray(inputs["b2revb"]).astype(np.int64)

    fbp = np.zeros((B, H), np.float32)
    fbp[:, :FD] = f_bonds
    fbT_full = np.ascontiguousarray(fbp.T)
    faT_full = np.ascontiguousarray(f_atoms.T)

    W = {k: np.asarray(inputs[k], np.float32) for k in
         ("Wq_atom", "Wk_atom", "Wv_atom", "Wi", "Wh", "Wq", "Wk", "Wv", "Wa",
          "Walpha_w", "Wo_w", "Wb_w")}
    wi_p = np.zeros((H, H), np.float32)
    wi_p[:FD, :] = W["Wi"]

    base = dict(
        wq_a=W["Wq_atom"], wk_a=W["Wk_atom"], wv_a=W["Wv_atom"],
        ln_g=np.asarray(inputs["ln_g"], np.float32).reshape(1, F),
        ln_b=np.asarray(inputs["ln_b"], np.float32).reshape(1, F),
        wi=wi_p, wh=W["Wh"], wq=W["Wq"], wk=W["Wk"], wv=W["Wv"], wa=W["Wa"],
        wal1=np.ascontiguousarray(W["Walpha_w"][:H]),
        wal2=np.ascontiguousarray(W["Walpha_w"][H:]),
        walb=np.asarray(inputs["Walpha_b"], np.float32).reshape(1, 1),
        wo_f=np.ascontiguousarray(W["Wo_w"][:F]),
        wo_m=np.ascontiguousarray(W["Wo_w"][F:]),
        wo_b=np.asarray(inputs["Wo_b"], np.float32).reshape(1, H),
        wb_w=W["Wb_w"],
        wb_b=np.asarray(inputs["Wb_b"], np.float32).reshape(1, H),
        ident=np.eye(128, dtype=np.float32),
        onesr=np.ones((1, 128), np.float32),
        onesc=np.ones((128, 1), np.float32),
    )

    in_maps = []
    for c in range(NC):
        bonds = np.arange(c * BS, (c + 1) * BS)
        # [7, BS]: row 0 = b2revb (subtracted), rows 1..6 = a2b[b2a]
        terms = np.stack([b2revb[bonds]]
                         + [a2b[b2a[bonds], j] for j in range(MNB)])
        nb2 = np.ascontiguousarray(
            terms.reshape(7, BS // 128, 128).transpose(2, 1, 0)).astype(np.int32)
        atoms = np.arange(c * AS, (c + 1) * AS)
        ro = np.stack([a2b[atoms, j] for j in range(MNB)])  # [6, AS]
        ro2 = np.ascontiguousarray(
            ro.reshape(MNB, AS // 128, 128).transpose(2, 1, 0)).astype(np.int32)
        m = dict(base)
        m["fbT"] = np.ascontiguousarray(fbT_full[:, bonds])
        m["faT"] = np.ascontiguousarray(faT_full[:, atoms])
        m["fa"] = np.ascontiguousarray(f_atoms[atoms])
        m["nb_idx"] = nb2
        m["ro_idx"] = ro2
        in_maps.append(m)
    return in_maps


def kernel(**inputs):
    if "nc" not in _CACHE:
        _CACHE["nc"] = _build()
    nc = _CACHE["nc"]
    in_maps = _host_prepare(inputs)
    res = run_bass_kernel_spmd(nc, in_maps, core_ids=list(range(NC)))
    out = np.concatenate([res.results[c]["mol_out"] for c in range(NC)], 0)
    return np.ascontiguousarray(out.astype(np.float32))
